# revision 3
# baseline (speedup 1.0000x reference)
"""Multi-head attention (dense transformer block) on 8 TRN2 NeuronCores. v2

Sharding: 8 cores = 4 batches x 2 head-halves (as v1).
  core c: batch b = c // 2, head half H = c % 2 (heads H*8 .. H*8+8).
  Host sums core pairs; bias folded into the even core of each pair.

v2 datapath is bf16 end-to-end (x, Wq/Wk/Wv/Wo in bf16; psum f32):
  1. QK projections -> psum f32 -> DVE evac to QT/KT bf16 [d, s].
     V projection -> psum [s, d] -> vp bf16 [k, (si h d|1)] with a ones
     column per head (rowsum trick).
  2. scores: per (head, qj, ki): psum[k=128, 1024] = K^T x Q chunks,
     ACT exp (scale=1/8) -> et bf16 [128, 1024].
  3. attn@V in [q, d] orientation: stationary = et q-slice [k=128, q=128]
     (full PE utilization), moving = vp [k=128, 65]: out psum [q, 65]
     accumulated over ki; col 64 = softmax denominator (per-partition).
     Normalize = one DVE tensor_scalar divide; PE-transpose [q,64]->[64,q]
     via identity into outT rows (sub*64..) - no gpsimd broadcast, no
     cross-partition DMA staging.
  4. final: out[s,e] = sum_g outT[g]^T @ Wo[g]; bias added during the
     DVE psum evacuation (tensor_tensor add) instead of a K=1 matmul.

Engines consume their queues in order, so projection/final work is
emitted *woven between* attention ki-steps (pop_filler) - the exp chain
is ACT-bound and the PE would otherwise idle ~350ns per ki-step.
"""

from collections import deque

import numpy as np

EMBED = 1024
HEADS = 16
HEAD_DIM = 64
SEQ = 2048
BATCH = 4
N_CORES = 8

LOCAL_HEADS = 8
N_GROUPS = 4
WCOLS = LOCAL_HEADS * HEAD_DIM  # 512

P = 128
NS = SEQ // P    # 16
NE = EMBED // P  # 8
VB = HEAD_DIM + 1  # 65

_cache = {}


def _emit(nc, tc, tile, mybir, make_identity, d):
    f32 = mybir.dt.float32
    bf16 = mybir.dt.bfloat16
    EXP = mybir.ActivationFunctionType.Exp
    DIV = mybir.AluOpType.divide

    with (
        tc.tile_pool(name="const", bufs=1) as const_pool,
        tc.tile_pool(name="xt", bufs=1) as xt_pool,
        tc.tile_pool(name="v", bufs=2) as v_pool,
        tc.tile_pool(name="qk", bufs=2) as qk_pool,
        tc.tile_pool(name="wst", bufs=1) as wst_pool,
        tc.tile_pool(name="ps_s", bufs=2, space="PSUM") as ps_s,
        tc.tile_pool(name="ps_p", bufs=2, space="PSUM") as ps_p,
        tc.tile_pool(name="ps_o", bufs=1, space="PSUM") as ps_o,
    ):
        def load_wv(half):
            wvt = wst_pool.tile([P, NE * 256], bf16, tag="wv", bufs=2, name="wvt")
            wv_v = d["wv"][:].rearrange("(e p) c -> p e c", e=NE, p=P)
            nc.sync.dma_start(
                out=wvt[:].rearrange("p (e c) -> p e c", e=NE, c=256),
                in_=wv_v[:, :, half * 256:(half + 1) * 256],
            )
            return wvt

        def load_wqk(name, g):
            wt = wst_pool.tile([P, NE * P], bf16, tag="wqk", bufs=4, name="wqk")
            w_v = d[name][:].rearrange("(e p) c -> p e c", e=NE, p=P)
            nc.sync.dma_start(
                out=wt[:].rearrange("p (e c) -> p e c", e=NE, c=P),
                in_=w_v[:, :, g * P:(g + 1) * P],
            )
            return [wt[:, ei * P:(ei + 1) * P] for ei in range(NE)]

        # DMA queue order = need order: the g0 QK weights gate the first
        # scores, then the first xT s-slab, then the V weights. One DMA
        # per slab: each dma_start costs ~650ns of serialized DGE queue
        # time, so few big transfers beat many small ones.
        # xt layout is sj-major (sj, ei, 512) so each per-sj DMA writes one
        # FLAT 2-d span (a 3-d strided write region defeats subtile dep
        # tracking -> readers race the DMA). All reads stay within one sj.
        wqk_pre = {"wq": load_wqk("wq", 0), "wk": load_wqk("wk", 0)}
        xt_big = xt_pool.tile([P, NE * SEQ], bf16, tag="xt", name="xt_big")
        xt_in = d["xt"][:].rearrange("(e p) s -> p e s", e=NE, p=P)

        def load_xt_sj(sj):
            nc.sync.dma_start(
                out=xt_big[:, sj * NE * 512:(sj + 1) * NE * 512],
                in_=xt_in[:, :, sj * 512:(sj + 1) * 512],
            )

        load_xt_sj(0)
        wvt_pre = load_wv(0)
        for sj in range(1, 4):
            load_xt_sj(sj)

        def xt_blk(ei, s0, slen):
            sj, off = s0 // 512, s0 % 512
            base = (sj * NE + ei) * 512 + off
            return xt_big[:, base: base + slen]

        ones128 = const_pool.tile([P, P], bf16, tag="ones", name="ones128")
        nc.gpsimd.memset(ones128[:], 1.0)
        ident = const_pool.tile([P, P], bf16, tag="ident", name="ident")
        make_identity(nc, ident[:])
        # warm the ACT exp table set during the DMA-bound startup
        warmf = const_pool.tile([1, 1], f32, tag="warmf", name="warmf")
        warm = const_pool.tile([1, 1], f32, tag="warm", name="warm")
        nc.vector.tensor_copy(warmf[:], ones128[0:1, 0:1])
        nc.scalar.activation(warm[:], warmf[:], EXP)
        # bias broadcast to all partitions (zeros on odd cores)
        bo_sb = const_pool.tile([1, EMBED], f32, tag="bo", name="bo_sb")
        nc.sync.dma_start(out=bo_sb[:], in_=d["bo"][:])
        bias_bc = const_pool.tile([P, EMBED], f32, tag="biasbc", name="bias_bc")
        nc.gpsimd.partition_broadcast(bias_bc[:], bo_sb[:])

        with (
            tc.tile_pool(name="et", bufs=6) as et_pool,
            tc.tile_pool(name="oq", bufs=3) as oq_pool,
            tc.tile_pool(name="outt", bufs=1) as outt_pool,
            tc.tile_pool(name="fin", bufs=4) as fin_pool,
        ):
            outt_tiles = [
                outt_pool.tile([P, SEQ], bf16, tag=f"outt{g}", name=f"outt{g}")
                for g in range(N_GROUPS)
            ]

            # vp: [128, NS*4*VB]; s-chunk si at si*4*VB, head (h%4) at h*VB;
            # col 64 of each head block is ones (rowsum trick).
            vp_tiles = []
            qkt_by_g = {}
            wo_tiles = []

            filler_q = deque()  # items: (tag, fn); tag marks the op's
            done_tags = set()   # completion point for ensure_tag()

            def pop_filler(n=1):
                for _ in range(n):
                    if filler_q:
                        tag, fn = filler_q.popleft()
                        fn()
                        if tag is not None:
                            done_tags.add(tag)

            def drain_fillers():
                pop_filler(len(filler_q))

            def ensure_tag(tag):
                while tag not in done_tags and filler_q:
                    pop_filler(1)

            def queue_ops(ops, tag=None):
                # tag attaches to the LAST op of the block
                for op in ops[:-1]:
                    filler_q.append((None, op))
                filler_q.append((tag, op if False else ops[-1]))

            def new_vp():
                vpt = v_pool.tile([P, NS * 4 * VB], bf16, tag="vp", name="vpt")
                vp_tiles.append(vpt)
                vp_v4 = vpt[:].rearrange("p (s h b) -> p s h b", s=NS, h=4, b=VB)
                nc.vector.tensor_copy(
                    vp_v4[:, :, :, HEAD_DIM:HEAD_DIM + 1],
                    ones128[:, 0:NS * 4].rearrange(
                        "p (a b c) -> p a b c", a=NS, b=4, c=1
                    ),
                )
                return vpt

            # Fillers are micro-ops (~one instruction each) so weaving them
            # into the ki-steps never delays the next scores matmul by more
            # than ~200ns (a chunky filler starves the ACT exp stream).
            def vblock_ops(wvt, vpt, si):
                vp_v = vpt[:].rearrange("p (s h b) -> p s h b", s=NS, h=4, b=VB)
                st = {}

                def mm(ei):
                    def go():
                        if ei == 0:
                            st["pt"] = ps_p.tile([P, 512], f32, tag="p", name="pt")
                        nc.tensor.matmul(
                            st["pt"][:, 0:256],
                            xt_blk(ei, si * P, P),
                            wvt[:, ei * 256:(ei + 1) * 256],
                            start=(ei == 0),
                            stop=(ei == NE - 1),
                        )
                    return go

                def evac():
                    nc.vector.tensor_copy(
                        vp_v[:, si, :, 0:HEAD_DIM],
                        st["pt"][:, 0:256].rearrange(
                            "p (h b) -> p h b", h=4, b=HEAD_DIM
                        ),
                    )
                return [mm(ei) for ei in range(NE)] + [evac]

            def qkchunk_ops(wtiles, dst, sj):
                st = {}

                def mm(ei):
                    def go():
                        if ei == 0:
                            st["pt"] = ps_p.tile([P, 512], f32, tag="p", name="pt")
                        nc.tensor.matmul(
                            st["pt"][:, 0:512],
                            wtiles[ei],
                            xt_blk(ei, sj * 512, 512),
                            start=(ei == 0),
                            stop=(ei == NE - 1),
                        )
                    return go

                def evac():
                    nc.vector.tensor_copy(
                        dst[:, sj * 512:(sj + 1) * 512], st["pt"][:, 0:512]
                    )
                return [mm(ei) for ei in range(NE)] + [evac]

            def final_ops(si, tail=False):
                st = {}
                ops = []

                def mm(ej, c):
                    def go():
                        if c == 0:
                            if ej == 0:
                                st["ot"] = fin_pool.tile(
                                    [P, 1024], f32, tag="ot", name="ot"
                                )
                            if tail and (si + ej) % 2 == 1:
                                st[ej] = ps_s.tile([P, 1024], f32, tag="s", name="pt")
                            else:
                                st[ej] = ps_p.tile([P, 512], f32, tag="p", name="pt")
                        nc.tensor.matmul(
                            st[ej][:, 0:512],
                            outt_tiles[c][:, si * P:(si + 1) * P],
                            wo_tiles[c // 2][:, (c % 2) * 1024 + ej * 512:
                                             (c % 2) * 1024 + (ej + 1) * 512],
                            start=(c == 0),
                            stop=(c == 3),
                            skip_group_check=True,
                        )
                    return go

                def evac(ej):
                    def go():
                        nc.vector.tensor_tensor(
                            out=st["ot"][:, ej * 512:(ej + 1) * 512],
                            in0=st[ej][:, 0:512],
                            in1=bias_bc[:, ej * 512:(ej + 1) * 512],
                            op=mybir.AluOpType.add,
                        )
                    return go

                def dma():
                    nc.sync.dma_start(
                        out=d["out"][si * P:(si + 1) * P, :], in_=st["ot"][:]
                    )
                for ej in range(2):
                    ops.extend([mm(ej, c) for c in range(4)])
                    ops.append(evac(ej))
                ops.append(dma)
                return ops

            def run_all(ops):
                for op in ops:
                    op()

            def load_wo():
                wo_a = wst_pool.tile([P, SEQ], bf16, tag="wv", bufs=2, name="wo_a")
                wo_b = qk_pool.tile([P, SEQ], bf16, tag="wqt", name="wo_b")
                wo_tiles.extend([wo_a, wo_b])
                for j in range(2):
                    for jj in range(2):
                        c = 2 * j + jj
                        nc.sync.dma_start(
                            out=wo_tiles[j][:, jj * 1024:(jj + 1) * 1024],
                            in_=d["wo"][c * P:(c + 1) * P, :],
                        )

            def vp_blk(si, h):
                vpt = vp_tiles[h // 4]
                base = (si * 4 + (h % 4)) * VB
                return vpt[:, base: base + VB]

            # ---- prelude: g0 QT/KT direct (gates the exp stream), then
            # the first V' blocks; the rest of V' weaves in as fillers.
            vpt0 = new_vp()
            qkt0 = {}
            for name in ("wq", "wk"):
                dst = qk_pool.tile([P, SEQ], bf16, tag=f"{name}t", name=f"{name}t")
                qkt0[name] = dst
            qkt_by_g[0] = qkt0
            # All g0 projection work is queued (not run) in rough need
            # order; unit 0's ki-loop pulls it just-in-time via ensure_tag
            # so the exp stream starts ~20us earlier than a serial prelude.
            TN = {"wq": "qt", "wk": "kt"}

            def q_qk0(name, sj):
                queue_ops(
                    qkchunk_ops(wqk_pre[name], qkt0[name], sj),
                    tag=(TN[name], 0, sj),
                )

            def q_vp0(si):
                queue_ops(vblock_ops(wvt_pre, vpt0, si), tag=("vp", 0, si))

            q_qk0("wk", 0)
            q_qk0("wq", 0)
            q_qk0("wq", 1)
            for si in range(4):
                q_vp0(si)
            q_qk0("wk", 1)
            for si in range(4, 7):
                q_vp0(si)
            q_qk0("wk", 2)
            for si in range(7, 10):
                q_vp0(si)
            q_qk0("wk", 3)
            for si in range(10, NS):
                q_vp0(si)
            q_qk0("wq", 2)
            q_qk0("wq", 3)

            # ---- groups ------------------------------------------------
            # Unit epilogue is split: the 8 DVE divides are emitted at unit
            # end (freeing the po banks before the next unit's attnV), while
            # the PE transposes + outT copies defer to ki==2 of the next
            # unit so the PE never parks on the divide chain's latency.
            pending_tr = [None]

            def emit_divides(po_q):
                oqs = []
                for qi in range(8):
                    rec = oq_pool.tile([P, 1], f32, tag="rec", bufs=9, name="rec")
                    nc.vector.reciprocal(rec[:], po_q(qi)[:, HEAD_DIM:VB])
                    oq = oq_pool.tile(
                        [P, HEAD_DIM], bf16, tag="oq", bufs=9, name="oq"
                    )
                    oqs.append(oq)
                    nc.vector.tensor_scalar(
                        oq[:],
                        po_q(qi)[:, 0:HEAD_DIM],
                        rec[:],
                        None,
                        op0=mybir.AluOpType.mult,
                    )
                return oqs

            def mk_trs(g, sub, q0, oqs):
                rows = slice(sub * HEAD_DIM, (sub + 1) * HEAD_DIM)

                def go():
                    for half4 in range(2):
                        tr = ps_p.tile([P, 512], bf16, tag="p", name="tr")
                        for j in range(4):
                            nc.tensor.transpose(
                                tr[rows, j * P:(j + 1) * P],
                                oqs[half4 * 4 + j][:],
                                ident[:],
                            )
                        nc.vector.tensor_copy(
                            outt_tiles[g][rows,
                                          q0 + half4 * 512: q0 + (half4 + 1) * 512],
                            tr[rows, :],
                        )
                return go

            vpt1_holder = {}
            deferred = {g: [] for g in range(N_GROUPS)}
            for g in range(N_GROUPS):
                # Stage the NEXT group's startup-critical chunks (first kt,
                # the qj=0 qt chunks, first V' blocks) into this group's
                # filler stream; the rest of its projections defer into its
                # OWN phase and are pulled just-in-time via ensure_tag -
                # this balances filler inventory against each phase's
                # ACT-bound PE holes (g2/g3 otherwise starve while g0/g1
                # burst-drain).
                if g + 1 < N_GROUPS:
                    gn = g + 1
                    if gn % 2 == 0:
                        wvt_n = load_wv(1)
                        vpt1 = new_vp()
                        vpt1_holder["wvt"] = wvt_n
                        vpt1_holder["vpt"] = vpt1
                        for si in range(NS):
                            queue_ops(vblock_ops(wvt_n, vpt1, si), tag=("vp", 1, si))
                    qkt_n = {}
                    for name in ("wq", "wk"):
                        dst = qk_pool.tile(
                            [P, SEQ], bf16, tag=f"{name}t", name=f"{name}t"
                        )
                        qkt_n[name] = dst
                        wtiles = load_wqk(name, gn)
                        for sj in range(4):
                            queue_ops(
                                qkchunk_ops(wtiles, dst, sj),
                                tag=({"wq": "qt", "wk": "kt"}[name], gn, sj),
                            )
                    qkt_by_g[gn] = qkt_n
                else:
                    load_wo()

                qt = qkt_by_g[g]["wq"]
                kt = qkt_by_g[g]["wk"]

                # Group 3 runs q-half-major; final si 0..7 only become
                # emittable once both qj=0 norms are in the stream (unit
                # index 2), so their fillers are queued there.
                if g < 3:
                    hq = [(s, qj) for s in range(2) for qj in range(2)]
                else:
                    hq = [(s, qj) for qj in range(2) for s in range(2)]
                for u, (sub, qj) in enumerate(hq):
                    h = 2 * g + sub
                    dr = slice(sub * HEAD_DIM, (sub + 1) * HEAD_DIM)
                    q0 = qj * 1024
                    po_a = ps_o.tile([P, 4 * VB], f32, tag="o0", name="po_a")
                    po_b = ps_o.tile([P, 4 * VB], f32, tag="o1", name="po_b")

                    def po_q(qi, _a=po_a, _b=po_b):
                        t = _a if qi < 4 else _b
                        return t[:, (qi % 4) * VB:(qi % 4) * VB + VB]

                    ensure_tag(("qt", g, 2 * qj))
                    ensure_tag(("qt", g, 2 * qj + 1))
                    et_tiles = {}
                    for ki in range(NS):
                        ensure_tag(("kt", g, ki // 4))
                        sps = ps_s.tile([P, 1024], f32, tag="s", name="sps")
                        for hf in range(2):
                            nc.tensor.matmul(
                                sps[:, hf * 512:(hf + 1) * 512],
                                kt[dr, ki * P:(ki + 1) * P],
                                qt[dr, q0 + hf * 512: q0 + (hf + 1) * 512],
                                start=True,
                                stop=True,
                            )
                        et = et_pool.tile([P, 1024], bf16, tag="et", name="et")
                        et_tiles[ki] = et
                        nc.scalar.activation(et[:], sps[:], EXP, scale=1.0 / 8.0)
                        # attnV skewed two ki behind the exp stream so the
                        # in-order PE queue never parks on a pending exp
                        if ki >= 2:
                            ensure_tag(("vp", h // 4, ki - 2))
                            for qi in range(8):
                                # start=True arms zero-on-write for the WHOLE
                                # 2KB psum bank, so only the first region per
                                # bank may set it; the others zero-fill via
                                # the armed pending-zero on their first write.
                                nc.tensor.matmul(
                                    po_q(qi),
                                    et_tiles[ki - 2][:, qi * P:(qi + 1) * P],
                                    vp_blk(ki - 2, h),
                                    start=(ki == 2 and qi % 4 == 0),
                                    stop=False,
                                    skip_group_check=True,
                                )
                        if ki == 2:
                            if pending_tr[0] is not None:
                                pending_tr[0]()
                                pending_tr[0] = None
                            # final si 0..7 become emittable once both qj=0
                            # outT halves are in the stream (g3 unit 2)
                            if g == 3 and u == 2:
                                for si in range(8):
                                    queue_ops(final_ops(si))
                        pop_filler(3)
                    ensure_tag(("vp", h // 4, NS - 1))
                    for kl in (NS - 2, NS - 1):
                        for qi in range(8):
                            nc.tensor.matmul(
                                po_q(qi),
                                et_tiles[kl][:, qi * P:(qi + 1) * P],
                                vp_blk(kl, h),
                                start=False,
                                stop=(kl == NS - 1),
                                skip_group_check=True,
                            )
                    oqs = emit_divides(po_q)
                    pending_tr[0] = mk_trs(g, sub, q0, oqs)

                if g < 3:
                    drain_fillers()

            pending_tr[0]()
            drain_fillers()
            for si in range(8, NS):
                run_all(final_ops(si, tail=True))


def _build_nc():
    import concourse.mybir as mybir
    import concourse.tile as tile
    from concourse import bacc
    from concourse.masks import make_identity

    f32 = mybir.dt.float32
    bf16 = mybir.dt.bfloat16
    nc = bacc.Bacc(
        "TRN2", target_bir_lowering=False, debug=False, num_devices=N_CORES
    )
    d = {
        "xt": nc.dram_tensor("xt", [EMBED, SEQ], bf16, kind="ExternalInput"),
        "wq": nc.dram_tensor("wq", [EMBED, WCOLS], bf16, kind="ExternalInput"),
        "wk": nc.dram_tensor("wk", [EMBED, WCOLS], bf16, kind="ExternalInput"),
        "wv": nc.dram_tensor("wv", [EMBED, WCOLS], bf16, kind="ExternalInput"),
        "wo": nc.dram_tensor("wo", [WCOLS, EMBED], bf16, kind="ExternalInput"),
        "bo": nc.dram_tensor("bo", [1, EMBED], f32, kind="ExternalInput"),
        "out": nc.dram_tensor("out", [SEQ, EMBED], f32, kind="ExternalOutput"),
    }
    with tile.TileContext(nc) as tc:
        _emit(nc, tc, tile, mybir, make_identity, d)
    nc.compile()
    return nc


def _get_nc():
    if "nc" not in _cache:
        _cache["nc"] = _build_nc()
    return _cache["nc"]


def make_in_maps(x, Wq, Wk, Wv, Wo, bo):
    import ml_dtypes

    bfarr = lambda a: np.ascontiguousarray(
        np.asarray(a, np.float32).astype(ml_dtypes.bfloat16)
    )
    x = np.asarray(x, dtype=np.float32)
    bo = np.asarray(bo, dtype=np.float32)
    xts = [bfarr(x[b].T) for b in range(BATCH)]
    Wq = np.asarray(Wq, np.float32)
    Wk = np.asarray(Wk, np.float32)
    Wv = np.asarray(Wv, np.float32)
    Wo = np.asarray(Wo, np.float32)
    in_maps = []
    for c in range(N_CORES):
        b, H = c // 2, c % 2
        cs = slice(H * WCOLS, (H + 1) * WCOLS)
        bo_eff = bo if H == 0 else np.zeros_like(bo)
        in_maps.append({
            "xt": xts[b],
            "wq": bfarr(Wq[:, cs]),
            "wk": bfarr(Wk[:, cs]),
            "wv": bfarr(Wv[:, cs]),
            "wo": bfarr(Wo[cs, :]),
            "bo": np.ascontiguousarray(bo_eff.reshape(1, EMBED)),
        })
    return in_maps


def _get_runner():
    """Cached jitted SPMD callable (avoids per-call retrace)."""
    if "runner" in _cache:
        return _cache["runner"]
    import jax
    from jax.sharding import Mesh, NamedSharding, PartitionSpec
    from jax.experimental.shard_map import shard_map
    from concourse import mybir
    from concourse.bass2jax import (
        _bass_exec_p,
        install_neuronx_cc_hook,
        partition_id_tensor,
    )

    nc = _get_nc()
    install_neuronx_cc_hook()
    pname = nc.partition_id_tensor.name if nc.partition_id_tensor else None
    in_names, out_names, out_avals, zeros = [], [], [], []
    for alloc in nc.m.functions[0].allocations:
        if not isinstance(alloc, mybir.MemoryLocationSet):
            continue
        name = alloc.memorylocations[0].name
        if alloc.kind == "ExternalInput":
            if name != pname:
                in_names.append(name)
        elif alloc.kind == "ExternalOutput":
            shape = tuple(alloc.tensor_shape)
            dtype = mybir.dt.np(alloc.dtype)
            out_names.append(name)
            out_avals.append(jax.core.ShapedArray(shape, dtype))
            zeros.append(np.zeros(shape, dtype))
    names_all = in_names + out_names + ([pname] if pname else [])

    def _body(*args):
        operands = list(args)
        if pname is not None:
            operands.append(partition_id_tensor())
        return tuple(_bass_exec_p.bind(
            *operands,
            out_avals=tuple(out_avals),
            in_names=tuple(names_all),
            out_names=tuple(out_names),
            lowering_input_output_aliases=(),
            sim_require_finite=True,
            sim_require_nnan=True,
            nc=nc,
        ))

    devices = jax.devices()[:N_CORES]
    mesh = Mesh(np.asarray(devices), ("core",))
    nio = len(in_names) + len(out_names)
    sharded = jax.jit(
        shard_map(
            _body, mesh=mesh,
            in_specs=(PartitionSpec("core"),) * nio,
            out_specs=(PartitionSpec("core"),) * len(out_names),
            check_rep=False,
        ),
        keep_unused=True,
    )
    sh = NamedSharding(mesh, PartitionSpec("core"))
    zdev = [
        jax.device_put(np.zeros((N_CORES * z.shape[0], *z.shape[1:]), z.dtype), sh)
        for z in zeros
    ]
    _cache["runner"] = (sharded, in_names, out_names, out_avals, zdev, sh)
    return _cache["runner"]


def kernel(x, Wq, Wk, Wv, Wo, bo, trace=False):
    in_maps = make_in_maps(x, Wq, Wk, Wv, Wo, bo)
    try:
        import jax

        sharded, in_names, out_names, out_avals, zdev, sh = _get_runner()
        concat = [
            jax.device_put(
                np.concatenate([m[n] for m in in_maps], axis=0), sh
            )
            for n in in_names
        ]
        outs = sharded(*concat, *zdev)
        arr = np.asarray(outs[out_names.index("out")]).reshape(
            N_CORES, SEQ, EMBED
        )
        out = np.empty((BATCH, SEQ, EMBED), dtype=np.float32)
        for b in range(BATCH):
            out[b] = arr[2 * b] + arr[2 * b + 1]
        return out
    except Exception:
        from concourse.bass_utils import run_bass_kernel_spmd

        nc = _get_nc()
        res = run_bass_kernel_spmd(
            nc, in_maps, list(range(N_CORES)), trace=trace
        )
        _cache["last_result"] = res
        out = np.empty((BATCH, SEQ, EMBED), dtype=np.float32)
        for b in range(BATCH):
            out[b] = res.results[2 * b]["out"] + res.results[2 * b + 1]["out"]
        return out


# revision 4
# speedup vs baseline: 3.0022x; 3.0022x over previous
"""Multi-head attention (dense transformer block) on 8 TRN2 NeuronCores. v2

Sharding: 8 cores = 4 batches x 2 head-halves (as v1).
  core c: batch b = c // 2, head half H = c % 2 (heads H*8 .. H*8+8).
  Host sums core pairs; bias folded into the even core of each pair.

v2 datapath is bf16 end-to-end (x, Wq/Wk/Wv/Wo in bf16; psum f32):
  1. QK projections -> psum f32 -> DVE evac to QT/KT bf16 [d, s].
     V projection -> psum [s, d] -> vp bf16 [k, (si h d|1)] with a ones
     column per head (rowsum trick).
  2. scores: per (head, qj, ki): psum[k=128, 1024] = K^T x Q chunks,
     ACT exp (scale=1/8) -> et bf16 [128, 1024].
  3. attn@V in [q, d] orientation: stationary = et q-slice [k=128, q=128]
     (full PE utilization), moving = vp [k=128, 65]: out psum [q, 65]
     accumulated over ki; col 64 = softmax denominator (per-partition).
     Normalize = one DVE tensor_scalar divide; PE-transpose [q,64]->[64,q]
     via identity into outT rows (sub*64..) - no gpsimd broadcast, no
     cross-partition DMA staging.
  4. final: out[s,e] = sum_g outT[g]^T @ Wo[g]; bias added during the
     DVE psum evacuation (tensor_tensor add) instead of a K=1 matmul.

Engines consume their queues in order, so projection/final work is
emitted *woven between* attention ki-steps (pop_filler) - the exp chain
is ACT-bound and the PE would otherwise idle ~350ns per ki-step.
"""

from collections import deque

import numpy as np

EMBED = 1024
HEADS = 16
HEAD_DIM = 64
SEQ = 2048
BATCH = 4
N_CORES = 8

LOCAL_HEADS = 8
N_GROUPS = 4
WCOLS = LOCAL_HEADS * HEAD_DIM  # 512

P = 128
NS = SEQ // P    # 16
NE = EMBED // P  # 8
VB = HEAD_DIM + 1  # 65

_cache = {}


def _emit(nc, tc, tile, mybir, make_identity, d):
    f32 = mybir.dt.float32
    bf16 = mybir.dt.bfloat16
    EXP = mybir.ActivationFunctionType.Exp
    DIV = mybir.AluOpType.divide

    with (
        tc.tile_pool(name="const", bufs=1) as const_pool,
        tc.tile_pool(name="xt", bufs=1) as xt_pool,
        tc.tile_pool(name="v", bufs=2) as v_pool,
        tc.tile_pool(name="qk", bufs=2) as qk_pool,
        tc.tile_pool(name="wst", bufs=1) as wst_pool,
        tc.tile_pool(name="ps_s", bufs=2, space="PSUM") as ps_s,
        tc.tile_pool(name="ps_p", bufs=2, space="PSUM") as ps_p,
        tc.tile_pool(name="ps_o", bufs=1, space="PSUM") as ps_o,
    ):
        def load_wv(half):
            wvt = wst_pool.tile([P, NE * 256], bf16, tag="wv", bufs=2, name="wvt")
            wv_v = d["wv"][:].rearrange("(e p) c -> p e c", e=NE, p=P)
            nc.sync.dma_start(
                out=wvt[:].rearrange("p (e c) -> p e c", e=NE, c=256),
                in_=wv_v[:, :, half * 256:(half + 1) * 256],
            )
            return wvt

        def load_wqk(name, g):
            wt = wst_pool.tile([P, NE * P], bf16, tag="wqk", bufs=4, name="wqk")
            w_v = d[name][:].rearrange("(e p) c -> p e c", e=NE, p=P)
            nc.sync.dma_start(
                out=wt[:].rearrange("p (e c) -> p e c", e=NE, c=P),
                in_=w_v[:, :, g * P:(g + 1) * P],
            )
            return [wt[:, ei * P:(ei + 1) * P] for ei in range(NE)]

        # DMA queue order = need order: the g0 QK weights gate the first
        # scores, then the first xT s-slab, then the V weights. One DMA
        # per slab: each dma_start costs ~650ns of serialized DGE queue
        # time, so few big transfers beat many small ones.
        # xt layout is sj-major (sj, ei, 512) so each per-sj DMA writes one
        # FLAT 2-d span (a 3-d strided write region defeats subtile dep
        # tracking -> readers race the DMA). All reads stay within one sj.
        wqk_pre = {"wq": load_wqk("wq", 0), "wk": load_wqk("wk", 0)}
        xt_big = xt_pool.tile([P, NE * SEQ], bf16, tag="xt", name="xt_big")
        xt_in = d["xt"][:].rearrange("(e p) s -> p e s", e=NE, p=P)

        # sj0 is split into two flat half-slabs (ha, ei, 256) so the very
        # first projections gate on 0.5MB of DMA instead of 1MB.
        def load_xt_sj(sj):
            if sj == 0:
                for ha in range(2):
                    nc.sync.dma_start(
                        out=xt_big[:, ha * NE * 256:(ha + 1) * NE * 256],
                        in_=xt_in[:, :, ha * 256:(ha + 1) * 256],
                    )
                return
            nc.sync.dma_start(
                out=xt_big[:, sj * NE * 512:(sj + 1) * NE * 512],
                in_=xt_in[:, :, sj * 512:(sj + 1) * 512],
            )

        load_xt_sj(0)
        wvt_pre = load_wv(0)
        for sj in range(1, 4):
            load_xt_sj(sj)

        def xt_blk(ei, s0, slen):
            if s0 < 512:
                ha, off = divmod(s0, 256)
                assert off + slen <= 256, (s0, slen)
                base = (ha * NE + ei) * 256 + off
                return xt_big[:, base: base + slen]
            sj, off = s0 // 512, s0 % 512
            base = (sj * NE + ei) * 512 + off
            return xt_big[:, base: base + slen]

        ones128 = const_pool.tile([P, P], bf16, tag="ones", name="ones128")
        nc.gpsimd.memset(ones128[:], 1.0)
        ident = const_pool.tile([P, P], bf16, tag="ident", name="ident")
        make_identity(nc, ident[:])
        # warm the ACT exp table set during the DMA-bound startup
        warmf = const_pool.tile([1, 1], f32, tag="warmf", name="warmf")
        warm = const_pool.tile([1, 1], f32, tag="warm", name="warm")
        nc.vector.tensor_copy(warmf[:], ones128[0:1, 0:1])
        nc.scalar.activation(warm[:], warmf[:], EXP)
        # bias broadcast to all partitions (zeros on odd cores)
        bo_sb = const_pool.tile([1, EMBED], f32, tag="bo", name="bo_sb")
        nc.sync.dma_start(out=bo_sb[:], in_=d["bo"][:])
        bias_bc = const_pool.tile([P, EMBED], f32, tag="biasbc", name="bias_bc")
        nc.gpsimd.partition_broadcast(bias_bc[:], bo_sb[:])

        with (
            tc.tile_pool(name="et", bufs=6) as et_pool,
            tc.tile_pool(name="oq", bufs=3) as oq_pool,
            tc.tile_pool(name="outt", bufs=1) as outt_pool,
            tc.tile_pool(name="fin", bufs=4) as fin_pool,
        ):
            outt_tiles = [
                outt_pool.tile([P, SEQ], bf16, tag=f"outt{g}", name=f"outt{g}")
                for g in range(N_GROUPS)
            ]

            # vp: [128, NS*4*VB]; s-chunk si at si*4*VB, head (h%4) at h*VB;
            # col 64 of each head block is ones (rowsum trick).
            vp_tiles = []
            qkt_by_g = {}
            wo_tiles = []

            filler_q = deque()  # items: (tag, fn); tag marks the op's
            done_tags = set()   # completion point for ensure_tag()

            def pop_filler(n=1):
                for _ in range(n):
                    if filler_q:
                        tag, fn = filler_q.popleft()
                        fn()
                        if tag is not None:
                            done_tags.add(tag)

            def drain_fillers():
                pop_filler(len(filler_q))

            def ensure_tag(tag):
                while tag not in done_tags and filler_q:
                    pop_filler(1)

            def queue_ops(ops, tag=None):
                # tag attaches to the LAST op of the block
                for op in ops[:-1]:
                    filler_q.append((None, op))
                filler_q.append((tag, op if False else ops[-1]))

            def new_vp():
                vpt = v_pool.tile([P, NS * 4 * VB], bf16, tag="vp", name="vpt")
                vp_tiles.append(vpt)
                vp_v4 = vpt[:].rearrange("p (s h b) -> p s h b", s=NS, h=4, b=VB)
                nc.vector.tensor_copy(
                    vp_v4[:, :, :, HEAD_DIM:HEAD_DIM + 1],
                    ones128[:, 0:NS * 4].rearrange(
                        "p (a b c) -> p a b c", a=NS, b=4, c=1
                    ),
                )
                return vpt

            # Fillers are micro-ops (~one instruction each) so weaving them
            # into the ki-steps never delays the next scores matmul by more
            # than ~200ns (a chunky filler starves the ACT exp stream).
            def vblock_ops(wvt, vpt, si):
                vp_v = vpt[:].rearrange("p (s h b) -> p s h b", s=NS, h=4, b=VB)
                st = {}

                def mm(ei):
                    def go():
                        if ei == 0:
                            st["pt"] = ps_p.tile([P, 512], f32, tag="p", name="pt")
                        nc.tensor.matmul(
                            st["pt"][:, 0:256],
                            xt_blk(ei, si * P, P),
                            wvt[:, ei * 256:(ei + 1) * 256],
                            start=(ei == 0),
                            stop=(ei == NE - 1),
                        )
                    return go

                def evac():
                    nc.vector.tensor_copy(
                        vp_v[:, si, :, 0:HEAD_DIM],
                        st["pt"][:, 0:256].rearrange(
                            "p (h b) -> p h b", h=4, b=HEAD_DIM
                        ),
                    )
                return [mm(ei) for ei in range(NE)] + [evac]

            def qkchunk_ops(wtiles, dst, sj):
                # sj0 reads the split xt half-slabs: two 256-wide moving
                # passes per ei (start=True once arms the bank; the other
                # regions zero-fill via pending-zero on first write).
                st = {}
                pieces = [(0, 256), (256, 256)] if sj == 0 else [(0, 512)]

                def mm(ei, off, w):
                    def go():
                        if ei == 0 and off == 0:
                            st["pt"] = ps_p.tile([P, 512], f32, tag="p", name="pt")
                        nc.tensor.matmul(
                            st["pt"][:, off:off + w],
                            wtiles[ei],
                            xt_blk(ei, sj * 512 + off, w),
                            start=(ei == 0 and off == 0),
                            stop=(ei == NE - 1 and off + w == 512),
                            skip_group_check=True,
                        )
                    return go

                def evac():
                    nc.vector.tensor_copy(
                        dst[:, sj * 512:(sj + 1) * 512], st["pt"][:, 0:512]
                    )
                ops = []
                for off, w in pieces:
                    ops.extend(mm(ei, off, w) for ei in range(NE))
                return ops + [evac]

            def final_ops(si, tail=False):
                st = {}
                ops = []

                def mm(ej, c):
                    def go():
                        if c == 0:
                            if ej == 0:
                                st["ot"] = fin_pool.tile(
                                    [P, 1024], f32, tag="ot", name="ot"
                                )
                            if tail and (si + ej) % 2 == 1:
                                st[ej] = ps_s.tile([P, 1024], f32, tag="s", name="pt")
                            else:
                                st[ej] = ps_p.tile([P, 512], f32, tag="p", name="pt")
                        nc.tensor.matmul(
                            st[ej][:, 0:512],
                            outt_tiles[c][:, si * P:(si + 1) * P],
                            wo_tiles[c // 2][:, (c % 2) * 1024 + ej * 512:
                                             (c % 2) * 1024 + (ej + 1) * 512],
                            start=(c == 0),
                            stop=(c == 3),
                            skip_group_check=True,
                        )
                    return go

                def evac(ej):
                    def go():
                        nc.vector.tensor_tensor(
                            out=st["ot"][:, ej * 512:(ej + 1) * 512],
                            in0=st[ej][:, 0:512],
                            in1=bias_bc[:, ej * 512:(ej + 1) * 512],
                            op=mybir.AluOpType.add,
                        )
                    return go

                def dma():
                    nc.sync.dma_start(
                        out=d["out"][si * P:(si + 1) * P, :], in_=st["ot"][:]
                    )
                for ej in range(2):
                    ops.extend([mm(ej, c) for c in range(4)])
                    ops.append(evac(ej))
                ops.append(dma)
                return ops

            def run_all(ops):
                for op in ops:
                    op()

            def load_wo():
                wo_a = wst_pool.tile([P, SEQ], bf16, tag="wv", bufs=2, name="wo_a")
                wo_b = qk_pool.tile([P, SEQ], bf16, tag="wqt", name="wo_b")
                wo_tiles.extend([wo_a, wo_b])
                for j in range(2):
                    for jj in range(2):
                        c = 2 * j + jj
                        nc.sync.dma_start(
                            out=wo_tiles[j][:, jj * 1024:(jj + 1) * 1024],
                            in_=d["wo"][c * P:(c + 1) * P, :],
                        )

            def vp_blk(si, h):
                vpt = vp_tiles[h // 4]
                base = (si * 4 + (h % 4)) * VB
                return vpt[:, base: base + VB]

            # ---- prelude: g0 QT/KT direct (gates the exp stream), then
            # the first V' blocks; the rest of V' weaves in as fillers.
            vpt0 = new_vp()
            qkt0 = {}
            for name in ("wq", "wk"):
                dst = qk_pool.tile([P, SEQ], bf16, tag=f"{name}t", name=f"{name}t")
                qkt0[name] = dst
            qkt_by_g[0] = qkt0
            # All g0 projection work is queued (not run) in rough need
            # order; unit 0's ki-loop pulls it just-in-time via ensure_tag
            # so the exp stream starts ~20us earlier than a serial prelude.
            TN = {"wq": "qt", "wk": "kt"}

            def q_qk0(name, sj):
                queue_ops(
                    qkchunk_ops(wqk_pre[name], qkt0[name], sj),
                    tag=(TN[name], 0, sj),
                )

            def q_vp0(si):
                queue_ops(vblock_ops(wvt_pre, vpt0, si), tag=("vp", 0, si))

            q_qk0("wk", 0)
            q_qk0("wq", 0)
            q_qk0("wq", 1)
            for si in range(4):
                q_vp0(si)
            q_qk0("wk", 1)
            for si in range(4, 7):
                q_vp0(si)
            q_qk0("wk", 2)
            for si in range(7, 10):
                q_vp0(si)
            q_qk0("wk", 3)
            for si in range(10, NS):
                q_vp0(si)
            q_qk0("wq", 2)
            q_qk0("wq", 3)

            # ---- groups ------------------------------------------------
            # Unit epilogue is split: the 8 DVE divides are emitted at unit
            # end (freeing the po banks before the next unit's attnV), while
            # the PE transposes + outT copies defer to ki==2 of the next
            # unit so the PE never parks on the divide chain's latency.
            pending_tr = [None]

            def emit_divides(po_q):
                oqs = []
                for qi in range(8):
                    rec = oq_pool.tile([P, 1], f32, tag="rec", bufs=9, name="rec")
                    nc.vector.reciprocal(rec[:], po_q(qi)[:, HEAD_DIM:VB])
                    oq = oq_pool.tile(
                        [P, HEAD_DIM], bf16, tag="oq", bufs=9, name="oq"
                    )
                    oqs.append(oq)
                    nc.vector.tensor_scalar(
                        oq[:],
                        po_q(qi)[:, 0:HEAD_DIM],
                        rec[:],
                        None,
                        op0=mybir.AluOpType.mult,
                    )
                return oqs

            def mk_trs(g, sub, q0, oqs):
                rows = slice(sub * HEAD_DIM, (sub + 1) * HEAD_DIM)

                def go():
                    for half4 in range(2):
                        tr = ps_p.tile([P, 512], bf16, tag="p", name="tr")
                        for j in range(4):
                            nc.tensor.transpose(
                                tr[rows, j * P:(j + 1) * P],
                                oqs[half4 * 4 + j][:],
                                ident[:],
                            )
                        nc.vector.tensor_copy(
                            outt_tiles[g][rows,
                                          q0 + half4 * 512: q0 + (half4 + 1) * 512],
                            tr[rows, :],
                        )
                return go

            vpt1_holder = {}
            deferred = {g: [] for g in range(N_GROUPS)}
            for g in range(N_GROUPS):
                # Stage the NEXT group's startup-critical chunks (first kt,
                # the qj=0 qt chunks, first V' blocks) into this group's
                # filler stream; the rest of its projections defer into its
                # OWN phase and are pulled just-in-time via ensure_tag -
                # this balances filler inventory against each phase's
                # ACT-bound PE holes (g2/g3 otherwise starve while g0/g1
                # burst-drain).
                if g + 1 < N_GROUPS:
                    gn = g + 1
                    if gn % 2 == 0:
                        wvt_n = load_wv(1)
                        vpt1 = new_vp()
                        vpt1_holder["wvt"] = wvt_n
                        vpt1_holder["vpt"] = vpt1
                        for si in range(NS):
                            queue_ops(vblock_ops(wvt_n, vpt1, si), tag=("vp", 1, si))
                    qkt_n = {}
                    for name in ("wq", "wk"):
                        dst = qk_pool.tile(
                            [P, SEQ], bf16, tag=f"{name}t", name=f"{name}t"
                        )
                        qkt_n[name] = dst
                        wtiles = load_wqk(name, gn)
                        for sj in range(4):
                            queue_ops(
                                qkchunk_ops(wtiles, dst, sj),
                                tag=({"wq": "qt", "wk": "kt"}[name], gn, sj),
                            )
                    qkt_by_g[gn] = qkt_n
                else:
                    load_wo()

                qt = qkt_by_g[g]["wq"]
                kt = qkt_by_g[g]["wk"]

                # Group 3 runs q-half-major; final si 0..7 only become
                # emittable once both qj=0 norms are in the stream (unit
                # index 2), so their fillers are queued there.
                if g < 3:
                    hq = [(s, qj) for s in range(2) for qj in range(2)]
                else:
                    hq = [(s, qj) for qj in range(2) for s in range(2)]
                for u, (sub, qj) in enumerate(hq):
                    h = 2 * g + sub
                    dr = slice(sub * HEAD_DIM, (sub + 1) * HEAD_DIM)
                    q0 = qj * 1024
                    po_a = ps_o.tile([P, 4 * VB], f32, tag="o0", name="po_a")
                    po_b = ps_o.tile([P, 4 * VB], f32, tag="o1", name="po_b")

                    def po_q(qi, _a=po_a, _b=po_b):
                        t = _a if qi < 4 else _b
                        return t[:, (qi % 4) * VB:(qi % 4) * VB + VB]

                    ensure_tag(("qt", g, 2 * qj))
                    ensure_tag(("qt", g, 2 * qj + 1))
                    et_tiles = {}
                    for ki in range(NS):
                        ensure_tag(("kt", g, ki // 4))
                        sps = ps_s.tile([P, 1024], f32, tag="s", name="sps")
                        for hf in range(2):
                            nc.tensor.matmul(
                                sps[:, hf * 512:(hf + 1) * 512],
                                kt[dr, ki * P:(ki + 1) * P],
                                qt[dr, q0 + hf * 512: q0 + (hf + 1) * 512],
                                start=True,
                                stop=True,
                            )
                        et = et_pool.tile([P, 1024], bf16, tag="et", name="et")
                        et_tiles[ki] = et
                        nc.scalar.activation(et[:], sps[:], EXP, scale=1.0 / 8.0)
                        # attnV skewed two ki behind the exp stream so the
                        # in-order PE queue never parks on a pending exp
                        if ki >= 2:
                            ensure_tag(("vp", h // 4, ki - 2))
                            for qi in range(8):
                                # start=True arms zero-on-write for the WHOLE
                                # 2KB psum bank, so only the first region per
                                # bank may set it; the others zero-fill via
                                # the armed pending-zero on their first write.
                                nc.tensor.matmul(
                                    po_q(qi),
                                    et_tiles[ki - 2][:, qi * P:(qi + 1) * P],
                                    vp_blk(ki - 2, h),
                                    start=(ki == 2 and qi % 4 == 0),
                                    stop=False,
                                    skip_group_check=True,
                                )
                        if ki == 2:
                            if pending_tr[0] is not None:
                                pending_tr[0]()
                                pending_tr[0] = None
                            # final si 0..7 become emittable once both qj=0
                            # outT halves are in the stream (g3 unit 2)
                            if g == 3 and u == 2:
                                for si in range(8):
                                    queue_ops(final_ops(si))
                        pop_filler(3)
                    ensure_tag(("vp", h // 4, NS - 1))
                    for kl in (NS - 2, NS - 1):
                        for qi in range(8):
                            nc.tensor.matmul(
                                po_q(qi),
                                et_tiles[kl][:, qi * P:(qi + 1) * P],
                                vp_blk(kl, h),
                                start=False,
                                stop=(kl == NS - 1),
                                skip_group_check=True,
                            )
                    oqs = emit_divides(po_q)
                    pending_tr[0] = mk_trs(g, sub, q0, oqs)

                if g < 3:
                    drain_fillers()

            pending_tr[0]()
            drain_fillers()
            for si in range(8, NS):
                run_all(final_ops(si, tail=True))


def _build_nc():
    import concourse.mybir as mybir
    import concourse.tile as tile
    from concourse import bacc
    from concourse.masks import make_identity

    f32 = mybir.dt.float32
    bf16 = mybir.dt.bfloat16
    nc = bacc.Bacc(
        "TRN2", target_bir_lowering=False, debug=False, num_devices=N_CORES
    )
    d = {
        "xt": nc.dram_tensor("xt", [EMBED, SEQ], bf16, kind="ExternalInput"),
        "wq": nc.dram_tensor("wq", [EMBED, WCOLS], bf16, kind="ExternalInput"),
        "wk": nc.dram_tensor("wk", [EMBED, WCOLS], bf16, kind="ExternalInput"),
        "wv": nc.dram_tensor("wv", [EMBED, WCOLS], bf16, kind="ExternalInput"),
        "wo": nc.dram_tensor("wo", [WCOLS, EMBED], bf16, kind="ExternalInput"),
        "bo": nc.dram_tensor("bo", [1, EMBED], f32, kind="ExternalInput"),
        "out": nc.dram_tensor("out", [SEQ, EMBED], f32, kind="ExternalOutput"),
    }
    with tile.TileContext(nc) as tc:
        _emit(nc, tc, tile, mybir, make_identity, d)
    nc.compile()
    return nc


def _get_nc():
    if "nc" not in _cache:
        _cache["nc"] = _build_nc()
    return _cache["nc"]


def make_in_maps(x, Wq, Wk, Wv, Wo, bo):
    import ml_dtypes

    bfarr = lambda a: np.ascontiguousarray(
        np.asarray(a, np.float32).astype(ml_dtypes.bfloat16)
    )
    x = np.asarray(x, dtype=np.float32)
    bo = np.asarray(bo, dtype=np.float32)
    xts = [bfarr(x[b].T) for b in range(BATCH)]
    Wq = np.asarray(Wq, np.float32)
    Wk = np.asarray(Wk, np.float32)
    Wv = np.asarray(Wv, np.float32)
    Wo = np.asarray(Wo, np.float32)
    in_maps = []
    for c in range(N_CORES):
        b, H = c // 2, c % 2
        cs = slice(H * WCOLS, (H + 1) * WCOLS)
        bo_eff = bo if H == 0 else np.zeros_like(bo)
        in_maps.append({
            "xt": xts[b],
            "wq": bfarr(Wq[:, cs]),
            "wk": bfarr(Wk[:, cs]),
            "wv": bfarr(Wv[:, cs]),
            "wo": bfarr(Wo[cs, :]),
            "bo": np.ascontiguousarray(bo_eff.reshape(1, EMBED)),
        })
    return in_maps


def _get_runner():
    """Cached jitted SPMD callable (avoids per-call retrace)."""
    if "runner" in _cache:
        return _cache["runner"]
    import jax
    from jax.sharding import Mesh, NamedSharding, PartitionSpec
    from jax.experimental.shard_map import shard_map
    from concourse import mybir
    from concourse.bass2jax import (
        _bass_exec_p,
        install_neuronx_cc_hook,
        partition_id_tensor,
    )

    nc = _get_nc()
    install_neuronx_cc_hook()
    pname = nc.partition_id_tensor.name if nc.partition_id_tensor else None
    in_names, out_names, out_avals, zeros = [], [], [], []
    for alloc in nc.m.functions[0].allocations:
        if not isinstance(alloc, mybir.MemoryLocationSet):
            continue
        name = alloc.memorylocations[0].name
        if alloc.kind == "ExternalInput":
            if name != pname:
                in_names.append(name)
        elif alloc.kind == "ExternalOutput":
            shape = tuple(alloc.tensor_shape)
            dtype = mybir.dt.np(alloc.dtype)
            out_names.append(name)
            out_avals.append(jax.core.ShapedArray(shape, dtype))
            zeros.append(np.zeros(shape, dtype))
    names_all = in_names + out_names + ([pname] if pname else [])

    def _body(*args):
        operands = list(args)
        if pname is not None:
            operands.append(partition_id_tensor())
        return tuple(_bass_exec_p.bind(
            *operands,
            out_avals=tuple(out_avals),
            in_names=tuple(names_all),
            out_names=tuple(out_names),
            lowering_input_output_aliases=(),
            sim_require_finite=True,
            sim_require_nnan=True,
            nc=nc,
        ))

    devices = jax.devices()[:N_CORES]
    mesh = Mesh(np.asarray(devices), ("core",))
    nio = len(in_names) + len(out_names)
    sharded = jax.jit(
        shard_map(
            _body, mesh=mesh,
            in_specs=(PartitionSpec("core"),) * nio,
            out_specs=(PartitionSpec("core"),) * len(out_names),
            check_rep=False,
        ),
        keep_unused=True,
    )
    sh = NamedSharding(mesh, PartitionSpec("core"))
    zdev = [
        jax.device_put(np.zeros((N_CORES * z.shape[0], *z.shape[1:]), z.dtype), sh)
        for z in zeros
    ]
    _cache["runner"] = (sharded, in_names, out_names, out_avals, zdev, sh)
    return _cache["runner"]


def kernel(x, Wq, Wk, Wv, Wo, bo, trace=False):
    in_maps = make_in_maps(x, Wq, Wk, Wv, Wo, bo)
    try:
        import jax

        sharded, in_names, out_names, out_avals, zdev, sh = _get_runner()
        concat = [
            jax.device_put(
                np.concatenate([m[n] for m in in_maps], axis=0), sh
            )
            for n in in_names
        ]
        outs = sharded(*concat, *zdev)
        arr = np.asarray(outs[out_names.index("out")]).reshape(
            N_CORES, SEQ, EMBED
        )
        out = np.empty((BATCH, SEQ, EMBED), dtype=np.float32)
        for b in range(BATCH):
            out[b] = arr[2 * b] + arr[2 * b + 1]
        return out
    except Exception:
        from concourse.bass_utils import run_bass_kernel_spmd

        nc = _get_nc()
        res = run_bass_kernel_spmd(
            nc, in_maps, list(range(N_CORES)), trace=trace
        )
        _cache["last_result"] = res
        out = np.empty((BATCH, SEQ, EMBED), dtype=np.float32)
        for b in range(BATCH):
            out[b] = res.results[2 * b]["out"] + res.results[2 * b + 1]["out"]
        return out


# revision 5
# speedup vs baseline: 3.0037x; 1.0005x over previous
"""Multi-head attention (dense transformer block) on 8 TRN2 NeuronCores. v2

Sharding: 8 cores = 4 batches x 2 head-halves (as v1).
  core c: batch b = c // 2, head half H = c % 2 (heads H*8 .. H*8+8).
  Host sums core pairs; bias folded into the even core of each pair.

v2 datapath is bf16 end-to-end (x, Wq/Wk/Wv/Wo in bf16; psum f32):
  1. QK projections -> psum f32 -> DVE evac to QT/KT bf16 [d, s].
     V projection -> psum [s, d] -> vp bf16 [k, (si h d|1)] with a ones
     column per head (rowsum trick).
  2. scores: per (head, qj, ki): psum[k=128, 1024] = K^T x Q chunks,
     ACT exp (scale=1/8) -> et bf16 [128, 1024].
  3. attn@V in [q, d] orientation: stationary = et q-slice [k=128, q=128]
     (full PE utilization), moving = vp [k=128, 65]: out psum [q, 65]
     accumulated over ki; col 64 = softmax denominator (per-partition).
     Normalize = one DVE tensor_scalar divide; PE-transpose [q,64]->[64,q]
     via identity into outT rows (sub*64..) - no gpsimd broadcast, no
     cross-partition DMA staging.
  4. final: out[s,e] = sum_g outT[g]^T @ Wo[g]; bias added during the
     DVE psum evacuation (tensor_tensor add) instead of a K=1 matmul.

Engines consume their queues in order, so projection/final work is
emitted *woven between* attention ki-steps (pop_filler) - the exp chain
is ACT-bound and the PE would otherwise idle ~350ns per ki-step.
"""

from collections import deque

import numpy as np

EMBED = 1024
HEADS = 16
HEAD_DIM = 64
SEQ = 2048
BATCH = 4
N_CORES = 8

LOCAL_HEADS = 8
N_GROUPS = 4
WCOLS = LOCAL_HEADS * HEAD_DIM  # 512

P = 128
NS = SEQ // P    # 16
NE = EMBED // P  # 8
VB = HEAD_DIM + 1  # 65

_cache = {}


def _emit(nc, tc, tile, mybir, make_identity, d):
    f32 = mybir.dt.float32
    bf16 = mybir.dt.bfloat16
    EXP = mybir.ActivationFunctionType.Exp
    DIV = mybir.AluOpType.divide

    with (
        tc.tile_pool(name="const", bufs=1) as const_pool,
        tc.tile_pool(name="xt", bufs=1) as xt_pool,
        tc.tile_pool(name="v", bufs=2) as v_pool,
        tc.tile_pool(name="qk", bufs=2) as qk_pool,
        tc.tile_pool(name="wst", bufs=1) as wst_pool,
        tc.tile_pool(name="ps_s", bufs=2, space="PSUM") as ps_s,
        tc.tile_pool(name="ps_p", bufs=2, space="PSUM") as ps_p,
        tc.tile_pool(name="ps_o", bufs=1, space="PSUM") as ps_o,
    ):
        def load_wv(half):
            wvt = wst_pool.tile([P, NE * 256], bf16, tag="wv", bufs=2, name="wvt")
            wv_v = d["wv"][:].rearrange("(e p) c -> p e c", e=NE, p=P)
            nc.sync.dma_start(
                out=wvt[:].rearrange("p (e c) -> p e c", e=NE, c=256),
                in_=wv_v[:, :, half * 256:(half + 1) * 256],
            )
            return wvt

        def load_wqk(name, g):
            wt = wst_pool.tile([P, NE * P], bf16, tag="wqk", bufs=4, name="wqk")
            w_v = d[name][:].rearrange("(e p) c -> p e c", e=NE, p=P)
            nc.sync.dma_start(
                out=wt[:].rearrange("p (e c) -> p e c", e=NE, c=P),
                in_=w_v[:, :, g * P:(g + 1) * P],
            )
            return [wt[:, ei * P:(ei + 1) * P] for ei in range(NE)]

        # DMA queue order = need order: the g0 QK weights gate the first
        # scores, then the first xT s-slab, then the V weights. One DMA
        # per slab: each dma_start costs ~650ns of serialized DGE queue
        # time, so few big transfers beat many small ones.
        # xt layout is sj-major (sj, ei, 512) so each per-sj DMA writes one
        # FLAT 2-d span (a 3-d strided write region defeats subtile dep
        # tracking -> readers race the DMA). All reads stay within one sj.
        wqk_pre = {"wq": load_wqk("wq", 0), "wk": load_wqk("wk", 0)}
        xt_big = xt_pool.tile([P, NE * SEQ], bf16, tag="xt", name="xt_big")
        xt_in = d["xt"][:].rearrange("(e p) s -> p e s", e=NE, p=P)

        # sj0 is split into two flat half-slabs (ha, ei, 256) so the very
        # first projections gate on 0.5MB of DMA instead of 1MB.
        def load_xt_sj(sj):
            if sj == 0:
                for ha in range(2):
                    nc.sync.dma_start(
                        out=xt_big[:, ha * NE * 256:(ha + 1) * NE * 256],
                        in_=xt_in[:, :, ha * 256:(ha + 1) * 256],
                    )
                return
            nc.sync.dma_start(
                out=xt_big[:, sj * NE * 512:(sj + 1) * NE * 512],
                in_=xt_in[:, :, sj * 512:(sj + 1) * 512],
            )

        load_xt_sj(0)
        wvt_pre = load_wv(0)
        for sj in range(1, 4):
            load_xt_sj(sj)

        def xt_blk(ei, s0, slen):
            if s0 < 512:
                ha, off = divmod(s0, 256)
                assert off + slen <= 256, (s0, slen)
                base = (ha * NE + ei) * 256 + off
                return xt_big[:, base: base + slen]
            sj, off = s0 // 512, s0 % 512
            base = (sj * NE + ei) * 512 + off
            return xt_big[:, base: base + slen]

        ones128 = const_pool.tile([P, P], bf16, tag="ones", name="ones128")
        nc.gpsimd.memset(ones128[:], 1.0)
        ident = const_pool.tile([P, P], bf16, tag="ident", name="ident")
        make_identity(nc, ident[:])
        # warm the ACT exp table set during the DMA-bound startup
        warmf = const_pool.tile([1, 1], f32, tag="warmf", name="warmf")
        warm = const_pool.tile([1, 1], f32, tag="warm", name="warm")
        nc.vector.tensor_copy(warmf[:], ones128[0:1, 0:1])
        nc.scalar.activation(warm[:], warmf[:], EXP)
        # bias broadcast to all partitions (zeros on odd cores)
        bo_sb = const_pool.tile([1, EMBED], f32, tag="bo", name="bo_sb")
        nc.sync.dma_start(out=bo_sb[:], in_=d["bo"][:])
        bias_bc = const_pool.tile([P, EMBED], f32, tag="biasbc", name="bias_bc")
        nc.gpsimd.partition_broadcast(bias_bc[:], bo_sb[:])

        with (
            tc.tile_pool(name="et", bufs=6) as et_pool,
            tc.tile_pool(name="oq", bufs=3) as oq_pool,
            tc.tile_pool(name="outt", bufs=1) as outt_pool,
            tc.tile_pool(name="fin", bufs=4) as fin_pool,
        ):
            outt_tiles = [
                outt_pool.tile([P, SEQ], bf16, tag=f"outt{g}", name=f"outt{g}")
                for g in range(N_GROUPS)
            ]

            # vp: [128, NS*4*VB]; s-chunk si at si*4*VB, head (h%4) at h*VB;
            # col 64 of each head block is ones (rowsum trick).
            vp_tiles = []
            qkt_by_g = {}
            wo_tiles = []

            filler_q = deque()  # items: (tag, fn); tag marks the op's
            done_tags = set()   # completion point for ensure_tag()

            def pop_filler(n=1):
                for _ in range(n):
                    if filler_q:
                        tag, fn = filler_q.popleft()
                        fn()
                        if tag is not None:
                            done_tags.add(tag)

            def drain_fillers():
                pop_filler(len(filler_q))

            def ensure_tag(tag):
                while tag not in done_tags and filler_q:
                    pop_filler(1)

            def queue_ops(ops, tag=None):
                # tag attaches to the LAST op of the block
                for op in ops[:-1]:
                    filler_q.append((None, op))
                filler_q.append((tag, op if False else ops[-1]))

            def new_vp():
                vpt = v_pool.tile([P, NS * 4 * VB], bf16, tag="vp", name="vpt")
                vp_tiles.append(vpt)
                vp_v4 = vpt[:].rearrange("p (s h b) -> p s h b", s=NS, h=4, b=VB)
                nc.vector.tensor_copy(
                    vp_v4[:, :, :, HEAD_DIM:HEAD_DIM + 1],
                    ones128[:, 0:NS * 4].rearrange(
                        "p (a b c) -> p a b c", a=NS, b=4, c=1
                    ),
                )
                return vpt

            # Fillers are micro-ops (~one instruction each) so weaving them
            # into the ki-steps never delays the next scores matmul by more
            # than ~200ns (a chunky filler starves the ACT exp stream).
            def vblock_ops(wvt, vpt, si):
                vp_v = vpt[:].rearrange("p (s h b) -> p s h b", s=NS, h=4, b=VB)
                st = {}

                def mm(ei):
                    def go():
                        if ei == 0:
                            st["pt"] = ps_p.tile([P, 512], f32, tag="p", name="pt")
                        nc.tensor.matmul(
                            st["pt"][:, 0:256],
                            xt_blk(ei, si * P, P),
                            wvt[:, ei * 256:(ei + 1) * 256],
                            start=(ei == 0),
                            stop=(ei == NE - 1),
                        )
                    return go

                def evac():
                    nc.vector.tensor_copy(
                        vp_v[:, si, :, 0:HEAD_DIM],
                        st["pt"][:, 0:256].rearrange(
                            "p (h b) -> p h b", h=4, b=HEAD_DIM
                        ),
                    )
                return [mm(ei) for ei in range(NE)] + [evac]

            def qkchunk_ops(wtiles, dst, sj):
                # sj0 reads the split xt half-slabs: two 256-wide moving
                # passes per ei (start=True once arms the bank; the other
                # regions zero-fill via pending-zero on first write).
                st = {}
                pieces = [(0, 256), (256, 256)] if sj == 0 else [(0, 512)]

                def mm(ei, off, w):
                    def go():
                        if ei == 0 and off == 0:
                            st["pt"] = ps_p.tile([P, 512], f32, tag="p", name="pt")
                        nc.tensor.matmul(
                            st["pt"][:, off:off + w],
                            wtiles[ei],
                            xt_blk(ei, sj * 512 + off, w),
                            start=(ei == 0 and off == 0),
                            stop=(ei == NE - 1 and off + w == 512),
                            skip_group_check=True,
                        )
                    return go

                def evac():
                    nc.vector.tensor_copy(
                        dst[:, sj * 512:(sj + 1) * 512], st["pt"][:, 0:512]
                    )
                ops = []
                for off, w in pieces:
                    ops.extend(mm(ei, off, w) for ei in range(NE))
                return ops + [evac]

            def final_ops(si, tail=False):
                st = {}
                ops = []

                def mm(ej, c):
                    def go():
                        if c == 0:
                            if ej == 0:
                                st["ot"] = fin_pool.tile(
                                    [P, 1024], f32, tag="ot", name="ot"
                                )
                            if tail and (si + ej) % 2 == 1:
                                st[ej] = ps_s.tile([P, 1024], f32, tag="s", name="pt")
                            else:
                                st[ej] = ps_p.tile([P, 512], f32, tag="p", name="pt")
                        nc.tensor.matmul(
                            st[ej][:, 0:512],
                            outt_tiles[c][:, si * P:(si + 1) * P],
                            wo_tiles[c // 2][:, (c % 2) * 1024 + ej * 512:
                                             (c % 2) * 1024 + (ej + 1) * 512],
                            start=(c == 0),
                            stop=(c == 3),
                            skip_group_check=True,
                        )
                    return go

                def evac(ej):
                    def go():
                        nc.vector.tensor_tensor(
                            out=st["ot"][:, ej * 512:(ej + 1) * 512],
                            in0=st[ej][:, 0:512],
                            in1=bias_bc[:, ej * 512:(ej + 1) * 512],
                            op=mybir.AluOpType.add,
                        )
                    return go

                def dma():
                    nc.sync.dma_start(
                        out=d["out"][si * P:(si + 1) * P, :], in_=st["ot"][:]
                    )
                for ej in range(2):
                    ops.extend([mm(ej, c) for c in range(4)])
                    ops.append(evac(ej))
                ops.append(dma)
                return ops

            def run_all(ops):
                for op in ops:
                    op()

            def load_wo():
                wo_a = wst_pool.tile([P, SEQ], bf16, tag="wv", bufs=2, name="wo_a")
                wo_b = qk_pool.tile([P, SEQ], bf16, tag="wqt", name="wo_b")
                wo_tiles.extend([wo_a, wo_b])
                for j in range(2):
                    for jj in range(2):
                        c = 2 * j + jj
                        nc.sync.dma_start(
                            out=wo_tiles[j][:, jj * 1024:(jj + 1) * 1024],
                            in_=d["wo"][c * P:(c + 1) * P, :],
                        )

            def vp_blk(si, h):
                vpt = vp_tiles[h // 4]
                base = (si * 4 + (h % 4)) * VB
                return vpt[:, base: base + VB]

            # ---- prelude: g0 QT/KT direct (gates the exp stream), then
            # the first V' blocks; the rest of V' weaves in as fillers.
            vpt0 = new_vp()
            qkt0 = {}
            for name in ("wq", "wk"):
                dst = qk_pool.tile([P, SEQ], bf16, tag=f"{name}t", name=f"{name}t")
                qkt0[name] = dst
            qkt_by_g[0] = qkt0
            # All g0 projection work is queued (not run) in rough need
            # order; unit 0's ki-loop pulls it just-in-time via ensure_tag
            # so the exp stream starts ~20us earlier than a serial prelude.
            TN = {"wq": "qt", "wk": "kt"}

            def q_qk0(name, sj):
                queue_ops(
                    qkchunk_ops(wqk_pre[name], qkt0[name], sj),
                    tag=(TN[name], 0, sj),
                )

            def q_vp0(si):
                queue_ops(vblock_ops(wvt_pre, vpt0, si), tag=("vp", 0, si))

            q_qk0("wk", 0)
            q_qk0("wq", 0)
            q_qk0("wq", 1)
            for si in range(4):
                q_vp0(si)
            q_qk0("wk", 1)
            for si in range(4, 7):
                q_vp0(si)
            q_qk0("wk", 2)
            for si in range(7, 10):
                q_vp0(si)
            q_qk0("wk", 3)
            for si in range(10, NS):
                q_vp0(si)
            q_qk0("wq", 2)
            q_qk0("wq", 3)

            # ---- groups ------------------------------------------------
            # Unit epilogue is split: the 8 DVE divides are emitted at unit
            # end (freeing the po banks before the next unit's attnV), while
            # the PE transposes + outT copies defer to ki==2 of the next
            # unit so the PE never parks on the divide chain's latency.
            pending_tr = [None]

            def emit_divides(po_q):
                oqs = []
                for qi in range(8):
                    rec = oq_pool.tile([P, 1], f32, tag="rec", bufs=9, name="rec")
                    nc.vector.reciprocal(rec[:], po_q(qi)[:, HEAD_DIM:VB])
                    oq = oq_pool.tile(
                        [P, HEAD_DIM], bf16, tag="oq", bufs=9, name="oq"
                    )
                    oqs.append(oq)
                    nc.vector.tensor_scalar(
                        oq[:],
                        po_q(qi)[:, 0:HEAD_DIM],
                        rec[:],
                        None,
                        op0=mybir.AluOpType.mult,
                    )
                return oqs

            def mk_trs(g, sub, q0, oqs):
                rows = slice(sub * HEAD_DIM, (sub + 1) * HEAD_DIM)

                def go():
                    for half4 in range(2):
                        tr = ps_p.tile([P, 512], bf16, tag="p", name="tr")
                        for j in range(4):
                            nc.tensor.transpose(
                                tr[rows, j * P:(j + 1) * P],
                                oqs[half4 * 4 + j][:],
                                ident[:],
                            )
                        nc.vector.tensor_copy(
                            outt_tiles[g][rows,
                                          q0 + half4 * 512: q0 + (half4 + 1) * 512],
                            tr[rows, :],
                        )
                return go

            vpt1_holder = {}
            deferred = {g: [] for g in range(N_GROUPS)}
            for g in range(N_GROUPS):
                # Stage the NEXT group's startup-critical chunks (first kt,
                # the qj=0 qt chunks, first V' blocks) into this group's
                # filler stream; the rest of its projections defer into its
                # OWN phase and are pulled just-in-time via ensure_tag -
                # this balances filler inventory against each phase's
                # ACT-bound PE holes (g2/g3 otherwise starve while g0/g1
                # burst-drain).
                if g + 1 < N_GROUPS:
                    gn = g + 1
                    if gn % 2 == 0:
                        wvt_n = load_wv(1)
                        vpt1 = new_vp()
                        vpt1_holder["wvt"] = wvt_n
                        vpt1_holder["vpt"] = vpt1
                        for si in range(NS):
                            queue_ops(vblock_ops(wvt_n, vpt1, si), tag=("vp", 1, si))
                    qkt_n = {}
                    for name in ("wq", "wk"):
                        dst = qk_pool.tile(
                            [P, SEQ], bf16, tag=f"{name}t", name=f"{name}t"
                        )
                        qkt_n[name] = dst
                        wtiles = load_wqk(name, gn)
                        for sj in range(4):
                            queue_ops(
                                qkchunk_ops(wtiles, dst, sj),
                                tag=({"wq": "qt", "wk": "kt"}[name], gn, sj),
                            )
                    qkt_by_g[gn] = qkt_n
                else:
                    load_wo()

                qt = qkt_by_g[g]["wq"]
                kt = qkt_by_g[g]["wk"]

                # Group 3 runs q-half-major; final si 0..7 only become
                # emittable once both qj=0 norms are in the stream (unit
                # index 2), so their fillers are queued there.
                if g < 3:
                    hq = [(s, qj) for s in range(2) for qj in range(2)]
                else:
                    hq = [(s, qj) for qj in range(2) for s in range(2)]
                for u, (sub, qj) in enumerate(hq):
                    h = 2 * g + sub
                    dr = slice(sub * HEAD_DIM, (sub + 1) * HEAD_DIM)
                    q0 = qj * 1024
                    po_a = ps_o.tile([P, 4 * VB], f32, tag="o0", name="po_a")
                    po_b = ps_o.tile([P, 4 * VB], f32, tag="o1", name="po_b")

                    def po_q(qi, _a=po_a, _b=po_b):
                        t = _a if qi < 4 else _b
                        return t[:, (qi % 4) * VB:(qi % 4) * VB + VB]

                    ensure_tag(("qt", g, 2 * qj))
                    ensure_tag(("qt", g, 2 * qj + 1))
                    et_tiles = {}
                    for ki in range(NS):
                        ensure_tag(("kt", g, ki // 4))
                        sps = ps_s.tile([P, 1024], f32, tag="s", name="sps")
                        for hf in range(2):
                            nc.tensor.matmul(
                                sps[:, hf * 512:(hf + 1) * 512],
                                kt[dr, ki * P:(ki + 1) * P],
                                qt[dr, q0 + hf * 512: q0 + (hf + 1) * 512],
                                start=True,
                                stop=True,
                            )
                        et = et_pool.tile([P, 1024], bf16, tag="et", name="et")
                        et_tiles[ki] = et
                        nc.scalar.activation(et[:], sps[:], EXP, scale=1.0 / 8.0)
                        # attnV skewed three ki behind the exp stream so the
                        # in-order PE queue never parks on a pending exp
                        if ki >= 3:
                            ensure_tag(("vp", h // 4, ki - 3))
                            for qi in range(8):
                                # start=True arms zero-on-write for the WHOLE
                                # 2KB psum bank, so only the first region per
                                # bank may set it; the others zero-fill via
                                # the armed pending-zero on their first write.
                                nc.tensor.matmul(
                                    po_q(qi),
                                    et_tiles[ki - 3][:, qi * P:(qi + 1) * P],
                                    vp_blk(ki - 3, h),
                                    start=(ki == 3 and qi % 4 == 0),
                                    stop=False,
                                    skip_group_check=True,
                                )
                        if ki == 2:
                            if pending_tr[0] is not None:
                                pending_tr[0]()
                                pending_tr[0] = None
                            # final si 0..7 become emittable once both qj=0
                            # outT halves are in the stream (g3 unit 2)
                            if g == 3 and u == 2:
                                for si in range(8):
                                    queue_ops(final_ops(si))
                        pop_filler(3)
                    ensure_tag(("vp", h // 4, NS - 1))
                    for kl in (NS - 3, NS - 2, NS - 1):
                        for qi in range(8):
                            nc.tensor.matmul(
                                po_q(qi),
                                et_tiles[kl][:, qi * P:(qi + 1) * P],
                                vp_blk(kl, h),
                                start=False,
                                stop=(kl == NS - 1),
                                skip_group_check=True,
                            )
                    oqs = emit_divides(po_q)
                    pending_tr[0] = mk_trs(g, sub, q0, oqs)

                if g < 3:
                    drain_fillers()

            pending_tr[0]()
            drain_fillers()
            for si in range(8, NS):
                run_all(final_ops(si, tail=True))


def _build_nc():
    import concourse.mybir as mybir
    import concourse.tile as tile
    from concourse import bacc
    from concourse.masks import make_identity

    f32 = mybir.dt.float32
    bf16 = mybir.dt.bfloat16
    nc = bacc.Bacc(
        "TRN2", target_bir_lowering=False, debug=False, num_devices=N_CORES
    )
    d = {
        "xt": nc.dram_tensor("xt", [EMBED, SEQ], bf16, kind="ExternalInput"),
        "wq": nc.dram_tensor("wq", [EMBED, WCOLS], bf16, kind="ExternalInput"),
        "wk": nc.dram_tensor("wk", [EMBED, WCOLS], bf16, kind="ExternalInput"),
        "wv": nc.dram_tensor("wv", [EMBED, WCOLS], bf16, kind="ExternalInput"),
        "wo": nc.dram_tensor("wo", [WCOLS, EMBED], bf16, kind="ExternalInput"),
        "bo": nc.dram_tensor("bo", [1, EMBED], f32, kind="ExternalInput"),
        "out": nc.dram_tensor("out", [SEQ, EMBED], f32, kind="ExternalOutput"),
    }
    with tile.TileContext(nc) as tc:
        _emit(nc, tc, tile, mybir, make_identity, d)
    nc.compile()
    return nc


def _get_nc():
    if "nc" not in _cache:
        _cache["nc"] = _build_nc()
    return _cache["nc"]


def make_in_maps(x, Wq, Wk, Wv, Wo, bo):
    import ml_dtypes

    bfarr = lambda a: np.ascontiguousarray(
        np.asarray(a, np.float32).astype(ml_dtypes.bfloat16)
    )
    x = np.asarray(x, dtype=np.float32)
    bo = np.asarray(bo, dtype=np.float32)
    xts = [bfarr(x[b].T) for b in range(BATCH)]
    Wq = np.asarray(Wq, np.float32)
    Wk = np.asarray(Wk, np.float32)
    Wv = np.asarray(Wv, np.float32)
    Wo = np.asarray(Wo, np.float32)
    in_maps = []
    for c in range(N_CORES):
        b, H = c // 2, c % 2
        cs = slice(H * WCOLS, (H + 1) * WCOLS)
        bo_eff = bo if H == 0 else np.zeros_like(bo)
        in_maps.append({
            "xt": xts[b],
            "wq": bfarr(Wq[:, cs]),
            "wk": bfarr(Wk[:, cs]),
            "wv": bfarr(Wv[:, cs]),
            "wo": bfarr(Wo[cs, :]),
            "bo": np.ascontiguousarray(bo_eff.reshape(1, EMBED)),
        })
    return in_maps


def _get_runner():
    """Cached jitted SPMD callable (avoids per-call retrace)."""
    if "runner" in _cache:
        return _cache["runner"]
    import jax
    from jax.sharding import Mesh, NamedSharding, PartitionSpec
    from jax.experimental.shard_map import shard_map
    from concourse import mybir
    from concourse.bass2jax import (
        _bass_exec_p,
        install_neuronx_cc_hook,
        partition_id_tensor,
    )

    nc = _get_nc()
    install_neuronx_cc_hook()
    pname = nc.partition_id_tensor.name if nc.partition_id_tensor else None
    in_names, out_names, out_avals, zeros = [], [], [], []
    for alloc in nc.m.functions[0].allocations:
        if not isinstance(alloc, mybir.MemoryLocationSet):
            continue
        name = alloc.memorylocations[0].name
        if alloc.kind == "ExternalInput":
            if name != pname:
                in_names.append(name)
        elif alloc.kind == "ExternalOutput":
            shape = tuple(alloc.tensor_shape)
            dtype = mybir.dt.np(alloc.dtype)
            out_names.append(name)
            out_avals.append(jax.core.ShapedArray(shape, dtype))
            zeros.append(np.zeros(shape, dtype))
    names_all = in_names + out_names + ([pname] if pname else [])

    def _body(*args):
        operands = list(args)
        if pname is not None:
            operands.append(partition_id_tensor())
        return tuple(_bass_exec_p.bind(
            *operands,
            out_avals=tuple(out_avals),
            in_names=tuple(names_all),
            out_names=tuple(out_names),
            lowering_input_output_aliases=(),
            sim_require_finite=True,
            sim_require_nnan=True,
            nc=nc,
        ))

    devices = jax.devices()[:N_CORES]
    mesh = Mesh(np.asarray(devices), ("core",))
    nio = len(in_names) + len(out_names)
    sharded = jax.jit(
        shard_map(
            _body, mesh=mesh,
            in_specs=(PartitionSpec("core"),) * nio,
            out_specs=(PartitionSpec("core"),) * len(out_names),
            check_rep=False,
        ),
        keep_unused=True,
    )
    sh = NamedSharding(mesh, PartitionSpec("core"))
    zdev = [
        jax.device_put(np.zeros((N_CORES * z.shape[0], *z.shape[1:]), z.dtype), sh)
        for z in zeros
    ]
    _cache["runner"] = (sharded, in_names, out_names, out_avals, zdev, sh)
    return _cache["runner"]


def kernel(x, Wq, Wk, Wv, Wo, bo, trace=False):
    in_maps = make_in_maps(x, Wq, Wk, Wv, Wo, bo)
    try:
        import jax

        sharded, in_names, out_names, out_avals, zdev, sh = _get_runner()
        concat = [
            jax.device_put(
                np.concatenate([m[n] for m in in_maps], axis=0), sh
            )
            for n in in_names
        ]
        outs = sharded(*concat, *zdev)
        arr = np.asarray(outs[out_names.index("out")]).reshape(
            N_CORES, SEQ, EMBED
        )
        out = np.empty((BATCH, SEQ, EMBED), dtype=np.float32)
        for b in range(BATCH):
            out[b] = arr[2 * b] + arr[2 * b + 1]
        return out
    except Exception:
        from concourse.bass_utils import run_bass_kernel_spmd

        nc = _get_nc()
        res = run_bass_kernel_spmd(
            nc, in_maps, list(range(N_CORES)), trace=trace
        )
        _cache["last_result"] = res
        out = np.empty((BATCH, SEQ, EMBED), dtype=np.float32)
        for b in range(BATCH):
            out[b] = res.results[2 * b]["out"] + res.results[2 * b + 1]["out"]
        return out


# revision 6
# speedup vs baseline: 3.0065x; 1.0009x over previous
"""Multi-head attention (dense transformer block) on 8 TRN2 NeuronCores. v2

Sharding: 8 cores = 4 batches x 2 head-halves (as v1).
  core c: batch b = c // 2, head half H = c % 2 (heads H*8 .. H*8+8).
  Host sums core pairs; bias folded into the even core of each pair.

v2 datapath is bf16 end-to-end (x, Wq/Wk/Wv/Wo in bf16; psum f32):
  1. QK projections -> psum f32 -> DVE evac to QT/KT bf16 [d, s].
     V projection -> psum [s, d] -> vp bf16 [k, (si h d|1)] with a ones
     column per head (rowsum trick).
  2. scores: per (head, qj, ki): psum[k=128, 1024] = K^T x Q chunks,
     ACT exp (scale=1/8) -> et bf16 [128, 1024].
  3. attn@V in [q, d] orientation: stationary = et q-slice [k=128, q=128]
     (full PE utilization), moving = vp [k=128, 65]: out psum [q, 65]
     accumulated over ki; col 64 = softmax denominator (per-partition).
     Normalize = one DVE tensor_scalar divide; PE-transpose [q,64]->[64,q]
     via identity into outT rows (sub*64..) - no gpsimd broadcast, no
     cross-partition DMA staging.
  4. final: out[s,e] = sum_g outT[g]^T @ Wo[g]; bias added during the
     DVE psum evacuation (tensor_tensor add) instead of a K=1 matmul.

Engines consume their queues in order, so projection/final work is
emitted *woven between* attention ki-steps (pop_filler) - the exp chain
is ACT-bound and the PE would otherwise idle ~350ns per ki-step.
"""

from collections import deque

import numpy as np

EMBED = 1024
HEADS = 16
HEAD_DIM = 64
SEQ = 2048
BATCH = 4
N_CORES = 8

LOCAL_HEADS = 8
N_GROUPS = 4
WCOLS = LOCAL_HEADS * HEAD_DIM  # 512

P = 128
NS = SEQ // P    # 16
NE = EMBED // P  # 8
VB = HEAD_DIM + 1  # 65

_cache = {}


def _emit(nc, tc, tile, mybir, make_identity, d):
    f32 = mybir.dt.float32
    bf16 = mybir.dt.bfloat16
    EXP = mybir.ActivationFunctionType.Exp
    DIV = mybir.AluOpType.divide

    with (
        tc.tile_pool(name="const", bufs=1) as const_pool,
        tc.tile_pool(name="xt", bufs=1) as xt_pool,
        tc.tile_pool(name="v", bufs=2) as v_pool,
        tc.tile_pool(name="qk", bufs=2) as qk_pool,
        tc.tile_pool(name="wst", bufs=1) as wst_pool,
        tc.tile_pool(name="ps_s", bufs=2, space="PSUM") as ps_s,
        tc.tile_pool(name="ps_p", bufs=2, space="PSUM") as ps_p,
        tc.tile_pool(name="ps_o", bufs=1, space="PSUM") as ps_o,
    ):
        def load_wv(half):
            wvt = wst_pool.tile([P, NE * 256], bf16, tag="wv", bufs=2, name="wvt")
            wv_v = d["wv"][:].rearrange("(e p) c -> p e c", e=NE, p=P)
            nc.sync.dma_start(
                out=wvt[:].rearrange("p (e c) -> p e c", e=NE, c=256),
                in_=wv_v[:, :, half * 256:(half + 1) * 256],
            )
            return wvt

        def load_wqk(name, g):
            wt = wst_pool.tile([P, NE * P], bf16, tag="wqk", bufs=4, name="wqk")
            w_v = d[name][:].rearrange("(e p) c -> p e c", e=NE, p=P)
            nc.sync.dma_start(
                out=wt[:].rearrange("p (e c) -> p e c", e=NE, c=P),
                in_=w_v[:, :, g * P:(g + 1) * P],
            )
            return [wt[:, ei * P:(ei + 1) * P] for ei in range(NE)]

        # DMA queue order = need order: the g0 QK weights gate the first
        # scores, then the first xT s-slab, then the V weights. One DMA
        # per slab: each dma_start costs ~650ns of serialized DGE queue
        # time, so few big transfers beat many small ones.
        # xt layout is sj-major (sj, ei, 512) so each per-sj DMA writes one
        # FLAT 2-d span (a 3-d strided write region defeats subtile dep
        # tracking -> readers race the DMA). All reads stay within one sj.
        wqk_pre = {"wq": load_wqk("wq", 0), "wk": load_wqk("wk", 0)}
        xt_big = xt_pool.tile([P, NE * SEQ], bf16, tag="xt", name="xt_big")
        xt_in = d["xt"][:].rearrange("(e p) s -> p e s", e=NE, p=P)

        # sj0 is split into two flat half-slabs (ha, ei, 256) so the very
        # first projections gate on 0.5MB of DMA instead of 1MB.
        def load_xt_sj(sj):
            if sj == 0:
                for ha in range(2):
                    nc.sync.dma_start(
                        out=xt_big[:, ha * NE * 256:(ha + 1) * NE * 256],
                        in_=xt_in[:, :, ha * 256:(ha + 1) * 256],
                    )
                return
            nc.sync.dma_start(
                out=xt_big[:, sj * NE * 512:(sj + 1) * NE * 512],
                in_=xt_in[:, :, sj * 512:(sj + 1) * 512],
            )

        load_xt_sj(0)
        wvt_pre = load_wv(0)
        for sj in range(1, 4):
            load_xt_sj(sj)

        def xt_blk(ei, s0, slen):
            if s0 < 512:
                ha, off = divmod(s0, 256)
                assert off + slen <= 256, (s0, slen)
                base = (ha * NE + ei) * 256 + off
                return xt_big[:, base: base + slen]
            sj, off = s0 // 512, s0 % 512
            base = (sj * NE + ei) * 512 + off
            return xt_big[:, base: base + slen]

        ones128 = const_pool.tile([P, P], bf16, tag="ones", name="ones128")
        nc.gpsimd.memset(ones128[:], 1.0)
        ident = const_pool.tile([P, P], bf16, tag="ident", name="ident")
        make_identity(nc, ident[:])
        # warm the ACT exp table set during the DMA-bound startup
        warmf = const_pool.tile([1, 1], f32, tag="warmf", name="warmf")
        warm = const_pool.tile([1, 1], f32, tag="warm", name="warm")
        nc.vector.tensor_copy(warmf[:], ones128[0:1, 0:1])
        nc.scalar.activation(warm[:], warmf[:], EXP)
        # bias broadcast to all partitions (zeros on odd cores)
        bo_sb = const_pool.tile([1, EMBED], f32, tag="bo", name="bo_sb")
        nc.sync.dma_start(out=bo_sb[:], in_=d["bo"][:])
        bias_bc = const_pool.tile([P, EMBED], f32, tag="biasbc", name="bias_bc")
        nc.gpsimd.partition_broadcast(bias_bc[:], bo_sb[:])

        with (
            tc.tile_pool(name="et", bufs=6) as et_pool,
            tc.tile_pool(name="oq", bufs=3) as oq_pool,
            tc.tile_pool(name="outt", bufs=1) as outt_pool,
            tc.tile_pool(name="fin", bufs=4) as fin_pool,
        ):
            outt_tiles = [
                outt_pool.tile([P, SEQ], bf16, tag=f"outt{g}", name=f"outt{g}")
                for g in range(N_GROUPS)
            ]

            # vp: [128, NS*4*VB]; s-chunk si at si*4*VB, head (h%4) at h*VB;
            # col 64 of each head block is ones (rowsum trick).
            vp_tiles = []
            qkt_by_g = {}
            wo_tiles = []

            filler_q = deque()  # items: (tag, fn); tag marks the op's
            done_tags = set()   # completion point for ensure_tag()

            def pop_filler(n=1):
                for _ in range(n):
                    if filler_q:
                        tag, fn = filler_q.popleft()
                        fn()
                        if tag is not None:
                            done_tags.add(tag)

            def drain_fillers():
                pop_filler(len(filler_q))

            def ensure_tag(tag):
                while tag not in done_tags and filler_q:
                    pop_filler(1)

            def queue_ops(ops, tag=None):
                # tag attaches to the LAST op of the block
                for op in ops[:-1]:
                    filler_q.append((None, op))
                filler_q.append((tag, op if False else ops[-1]))

            def new_vp():
                vpt = v_pool.tile([P, NS * 4 * VB], bf16, tag="vp", name="vpt")
                vp_tiles.append(vpt)
                vp_v4 = vpt[:].rearrange("p (s h b) -> p s h b", s=NS, h=4, b=VB)
                nc.vector.tensor_copy(
                    vp_v4[:, :, :, HEAD_DIM:HEAD_DIM + 1],
                    ones128[:, 0:NS * 4].rearrange(
                        "p (a b c) -> p a b c", a=NS, b=4, c=1
                    ),
                )
                return vpt

            # Fillers are micro-ops (~one instruction each) so weaving them
            # into the ki-steps never delays the next scores matmul by more
            # than ~200ns (a chunky filler starves the ACT exp stream).
            def vblock_ops(wvt, vpt, si):
                vp_v = vpt[:].rearrange("p (s h b) -> p s h b", s=NS, h=4, b=VB)
                st = {}

                def mm(ei):
                    def go():
                        if ei == 0:
                            st["pt"] = ps_p.tile([P, 512], f32, tag="p", name="pt")
                        nc.tensor.matmul(
                            st["pt"][:, 0:256],
                            xt_blk(ei, si * P, P),
                            wvt[:, ei * 256:(ei + 1) * 256],
                            start=(ei == 0),
                            stop=(ei == NE - 1),
                        )
                    return go

                def evac():
                    nc.vector.tensor_copy(
                        vp_v[:, si, :, 0:HEAD_DIM],
                        st["pt"][:, 0:256].rearrange(
                            "p (h b) -> p h b", h=4, b=HEAD_DIM
                        ),
                    )
                return [mm(ei) for ei in range(NE)] + [evac]

            def qkchunk_ops(wtiles, dst, sj):
                # sj0 reads the split xt half-slabs: two 256-wide moving
                # passes per ei (start=True once arms the bank; the other
                # regions zero-fill via pending-zero on first write).
                st = {}
                pieces = [(0, 256), (256, 256)] if sj == 0 else [(0, 512)]

                def mm(ei, off, w):
                    def go():
                        if ei == 0 and off == 0:
                            st["pt"] = ps_p.tile([P, 512], f32, tag="p", name="pt")
                        nc.tensor.matmul(
                            st["pt"][:, off:off + w],
                            wtiles[ei],
                            xt_blk(ei, sj * 512 + off, w),
                            start=(ei == 0 and off == 0),
                            stop=(ei == NE - 1 and off + w == 512),
                            skip_group_check=True,
                        )
                    return go

                def evac():
                    nc.vector.tensor_copy(
                        dst[:, sj * 512:(sj + 1) * 512], st["pt"][:, 0:512]
                    )
                ops = []
                for off, w in pieces:
                    ops.extend(mm(ei, off, w) for ei in range(NE))
                return ops + [evac]

            def final_ops(si, tail=False, force_s=False):
                st = {}
                ops = []

                def mm(ej, c):
                    def go():
                        if c == 0:
                            if ej == 0:
                                st["ot"] = fin_pool.tile(
                                    [P, 1024], f32, tag="ot", name="ot"
                                )
                            if force_s or (tail and (si + ej) % 2 == 1):
                                st[ej] = ps_s.tile([P, 1024], f32, tag="s", name="pt")
                            else:
                                st[ej] = ps_p.tile([P, 512], f32, tag="p", name="pt")
                        nc.tensor.matmul(
                            st[ej][:, 0:512],
                            outt_tiles[c][:, si * P:(si + 1) * P],
                            wo_tiles[c // 2][:, (c % 2) * 1024 + ej * 512:
                                             (c % 2) * 1024 + (ej + 1) * 512],
                            start=(c == 0),
                            stop=(c == 3),
                            skip_group_check=True,
                        )
                    return go

                def evac(ej):
                    def go():
                        nc.vector.tensor_tensor(
                            out=st["ot"][:, ej * 512:(ej + 1) * 512],
                            in0=st[ej][:, 0:512],
                            in1=bias_bc[:, ej * 512:(ej + 1) * 512],
                            op=mybir.AluOpType.add,
                        )
                    return go

                def dma():
                    nc.sync.dma_start(
                        out=d["out"][si * P:(si + 1) * P, :], in_=st["ot"][:]
                    )
                for ej in range(2):
                    ops.extend([mm(ej, c) for c in range(4)])
                    ops.append(evac(ej))
                ops.append(dma)
                return ops

            def run_all(ops):
                for op in ops:
                    op()

            def load_wo():
                wo_a = wst_pool.tile([P, SEQ], bf16, tag="wv", bufs=2, name="wo_a")
                wo_b = qk_pool.tile([P, SEQ], bf16, tag="wqt", name="wo_b")
                wo_tiles.extend([wo_a, wo_b])
                for j in range(2):
                    for jj in range(2):
                        c = 2 * j + jj
                        nc.sync.dma_start(
                            out=wo_tiles[j][:, jj * 1024:(jj + 1) * 1024],
                            in_=d["wo"][c * P:(c + 1) * P, :],
                        )

            def vp_blk(si, h):
                vpt = vp_tiles[h // 4]
                base = (si * 4 + (h % 4)) * VB
                return vpt[:, base: base + VB]

            # ---- prelude: g0 QT/KT direct (gates the exp stream), then
            # the first V' blocks; the rest of V' weaves in as fillers.
            vpt0 = new_vp()
            qkt0 = {}
            for name in ("wq", "wk"):
                dst = qk_pool.tile([P, SEQ], bf16, tag=f"{name}t", name=f"{name}t")
                qkt0[name] = dst
            qkt_by_g[0] = qkt0
            # All g0 projection work is queued (not run) in rough need
            # order; unit 0's ki-loop pulls it just-in-time via ensure_tag
            # so the exp stream starts ~20us earlier than a serial prelude.
            TN = {"wq": "qt", "wk": "kt"}

            def q_qk0(name, sj):
                queue_ops(
                    qkchunk_ops(wqk_pre[name], qkt0[name], sj),
                    tag=(TN[name], 0, sj),
                )

            def q_vp0(si):
                queue_ops(vblock_ops(wvt_pre, vpt0, si), tag=("vp", 0, si))

            q_qk0("wk", 0)
            q_qk0("wq", 0)
            q_qk0("wq", 1)
            for si in range(4):
                q_vp0(si)
            q_qk0("wk", 1)
            for si in range(4, 7):
                q_vp0(si)
            q_qk0("wk", 2)
            for si in range(7, 10):
                q_vp0(si)
            q_qk0("wk", 3)
            for si in range(10, NS):
                q_vp0(si)
            q_qk0("wq", 2)
            q_qk0("wq", 3)

            # ---- groups ------------------------------------------------
            # Unit epilogue is split: the 8 DVE divides are emitted at unit
            # end (freeing the po banks before the next unit's attnV), while
            # the PE transposes + outT copies defer to ki==2 of the next
            # unit so the PE never parks on the divide chain's latency.
            pending_tr = [None]

            def emit_divides(po_q):
                oqs = []
                for qi in range(8):
                    rec = oq_pool.tile([P, 1], f32, tag="rec", bufs=9, name="rec")
                    nc.vector.reciprocal(rec[:], po_q(qi)[:, HEAD_DIM:VB])
                    oq = oq_pool.tile(
                        [P, HEAD_DIM], bf16, tag="oq", bufs=9, name="oq"
                    )
                    oqs.append(oq)
                    nc.vector.tensor_scalar(
                        oq[:],
                        po_q(qi)[:, 0:HEAD_DIM],
                        rec[:],
                        None,
                        op0=mybir.AluOpType.mult,
                    )
                return oqs

            def mk_trs(g, sub, q0, oqs):
                rows = slice(sub * HEAD_DIM, (sub + 1) * HEAD_DIM)

                def go():
                    for half4 in range(2):
                        tr = ps_p.tile([P, 512], bf16, tag="p", name="tr")
                        for j in range(4):
                            nc.tensor.transpose(
                                tr[rows, j * P:(j + 1) * P],
                                oqs[half4 * 4 + j][:],
                                ident[:],
                            )
                        nc.vector.tensor_copy(
                            outt_tiles[g][rows,
                                          q0 + half4 * 512: q0 + (half4 + 1) * 512],
                            tr[rows, :],
                        )
                return go

            vpt1_holder = {}
            deferred = {g: [] for g in range(N_GROUPS)}
            for g in range(N_GROUPS):
                # Stage the NEXT group's startup-critical chunks (first kt,
                # the qj=0 qt chunks, first V' blocks) into this group's
                # filler stream; the rest of its projections defer into its
                # OWN phase and are pulled just-in-time via ensure_tag -
                # this balances filler inventory against each phase's
                # ACT-bound PE holes (g2/g3 otherwise starve while g0/g1
                # burst-drain).
                if g + 1 < N_GROUPS:
                    gn = g + 1
                    if gn % 2 == 0:
                        wvt_n = load_wv(1)
                        vpt1 = new_vp()
                        vpt1_holder["wvt"] = wvt_n
                        vpt1_holder["vpt"] = vpt1
                        for si in range(NS):
                            queue_ops(vblock_ops(wvt_n, vpt1, si), tag=("vp", 1, si))
                    qkt_n = {}
                    for name in ("wq", "wk"):
                        dst = qk_pool.tile(
                            [P, SEQ], bf16, tag=f"{name}t", name=f"{name}t"
                        )
                        qkt_n[name] = dst
                        wtiles = load_wqk(name, gn)
                        for sj in range(4):
                            queue_ops(
                                qkchunk_ops(wtiles, dst, sj),
                                tag=({"wq": "qt", "wk": "kt"}[name], gn, sj),
                            )
                    qkt_by_g[gn] = qkt_n
                else:
                    load_wo()

                qt = qkt_by_g[g]["wq"]
                kt = qkt_by_g[g]["wk"]

                # Group 3 runs q-half-major; final si 0..7 only become
                # emittable once both qj=0 norms are in the stream (unit
                # index 2), so their fillers are queued there.
                if g < 3:
                    hq = [(s, qj) for s in range(2) for qj in range(2)]
                else:
                    hq = [(s, qj) for qj in range(2) for s in range(2)]
                for u, (sub, qj) in enumerate(hq):
                    h = 2 * g + sub
                    dr = slice(sub * HEAD_DIM, (sub + 1) * HEAD_DIM)
                    q0 = qj * 1024
                    po_a = ps_o.tile([P, 4 * VB], f32, tag="o0", name="po_a")
                    po_b = ps_o.tile([P, 4 * VB], f32, tag="o1", name="po_b")

                    def po_q(qi, _a=po_a, _b=po_b):
                        t = _a if qi < 4 else _b
                        return t[:, (qi % 4) * VB:(qi % 4) * VB + VB]

                    ensure_tag(("qt", g, 2 * qj))
                    ensure_tag(("qt", g, 2 * qj + 1))
                    et_tiles = {}
                    for ki in range(NS):
                        ensure_tag(("kt", g, ki // 4))
                        sps = ps_s.tile([P, 1024], f32, tag="s", name="sps")
                        for hf in range(2):
                            nc.tensor.matmul(
                                sps[:, hf * 512:(hf + 1) * 512],
                                kt[dr, ki * P:(ki + 1) * P],
                                qt[dr, q0 + hf * 512: q0 + (hf + 1) * 512],
                                start=True,
                                stop=True,
                            )
                        et = et_pool.tile([P, 1024], bf16, tag="et", name="et")
                        et_tiles[ki] = et
                        nc.scalar.activation(et[:], sps[:], EXP, scale=1.0 / 8.0)
                        # attnV skewed three ki behind the exp stream so the
                        # in-order PE queue never parks on a pending exp
                        if ki >= 3:
                            ensure_tag(("vp", h // 4, ki - 3))
                            for qi in range(8):
                                # start=True arms zero-on-write for the WHOLE
                                # 2KB psum bank, so only the first region per
                                # bank may set it; the others zero-fill via
                                # the armed pending-zero on their first write.
                                nc.tensor.matmul(
                                    po_q(qi),
                                    et_tiles[ki - 3][:, qi * P:(qi + 1) * P],
                                    vp_blk(ki - 3, h),
                                    start=(ki == 3 and qi % 4 == 0),
                                    stop=False,
                                    skip_group_check=True,
                                )
                        if ki == 2:
                            if pending_tr[0] is not None:
                                pending_tr[0]()
                                pending_tr[0] = None
                            # final si 0..7 become emittable once both qj=0
                            # outT halves are in the stream (g3 unit 2)
                            if g == 3 and u == 2:
                                for si in range(8):
                                    queue_ops(final_ops(si))
                        pop_filler(3)
                    ensure_tag(("vp", h // 4, NS - 1))
                    for kl in (NS - 3, NS - 2, NS - 1):
                        for qi in range(8):
                            nc.tensor.matmul(
                                po_q(qi),
                                et_tiles[kl][:, qi * P:(qi + 1) * P],
                                vp_blk(kl, h),
                                start=False,
                                stop=(kl == NS - 1),
                                skip_group_check=True,
                            )
                    oqs = emit_divides(po_q)
                    pending_tr[0] = mk_trs(g, sub, q0, oqs)

                if g < 3:
                    drain_fillers()

            # The first two tail finals' c0..c2 accumulations depend only
            # on outT[0..2]; emitting them before the last unit's transposes
            # keeps the PE fed while the divide chain drains on DVE.
            # ops layout per si: [mm(ej0,c0..c3), evac0, mm(ej1,c0..c3),
            # evac1, dma]; only the c3 matmuls read outT[3], so c0..c2 of
            # both ej may run before the last transposes land.
            head_a = final_ops(8, tail=True, force_s=True)
            for op in head_a[0:3] + head_a[5:8]:
                op()
            pending_tr[0]()
            drain_fillers()
            for op in head_a[3:5] + head_a[8:11]:
                op()
            for si in range(9, NS):
                run_all(final_ops(si, tail=True))


def _build_nc():
    import concourse.mybir as mybir
    import concourse.tile as tile
    from concourse import bacc
    from concourse.masks import make_identity

    f32 = mybir.dt.float32
    bf16 = mybir.dt.bfloat16
    nc = bacc.Bacc(
        "TRN2", target_bir_lowering=False, debug=False, num_devices=N_CORES
    )
    d = {
        "xt": nc.dram_tensor("xt", [EMBED, SEQ], bf16, kind="ExternalInput"),
        "wq": nc.dram_tensor("wq", [EMBED, WCOLS], bf16, kind="ExternalInput"),
        "wk": nc.dram_tensor("wk", [EMBED, WCOLS], bf16, kind="ExternalInput"),
        "wv": nc.dram_tensor("wv", [EMBED, WCOLS], bf16, kind="ExternalInput"),
        "wo": nc.dram_tensor("wo", [WCOLS, EMBED], bf16, kind="ExternalInput"),
        "bo": nc.dram_tensor("bo", [1, EMBED], f32, kind="ExternalInput"),
        "out": nc.dram_tensor("out", [SEQ, EMBED], f32, kind="ExternalOutput"),
    }
    with tile.TileContext(nc) as tc:
        _emit(nc, tc, tile, mybir, make_identity, d)
    nc.compile()
    return nc


def _get_nc():
    if "nc" not in _cache:
        _cache["nc"] = _build_nc()
    return _cache["nc"]


def make_in_maps(x, Wq, Wk, Wv, Wo, bo):
    import ml_dtypes

    bfarr = lambda a: np.ascontiguousarray(
        np.asarray(a, np.float32).astype(ml_dtypes.bfloat16)
    )
    x = np.asarray(x, dtype=np.float32)
    bo = np.asarray(bo, dtype=np.float32)
    xts = [bfarr(x[b].T) for b in range(BATCH)]
    Wq = np.asarray(Wq, np.float32)
    Wk = np.asarray(Wk, np.float32)
    Wv = np.asarray(Wv, np.float32)
    Wo = np.asarray(Wo, np.float32)
    in_maps = []
    for c in range(N_CORES):
        b, H = c // 2, c % 2
        cs = slice(H * WCOLS, (H + 1) * WCOLS)
        bo_eff = bo if H == 0 else np.zeros_like(bo)
        in_maps.append({
            "xt": xts[b],
            "wq": bfarr(Wq[:, cs]),
            "wk": bfarr(Wk[:, cs]),
            "wv": bfarr(Wv[:, cs]),
            "wo": bfarr(Wo[cs, :]),
            "bo": np.ascontiguousarray(bo_eff.reshape(1, EMBED)),
        })
    return in_maps


def _get_runner():
    """Cached jitted SPMD callable (avoids per-call retrace)."""
    if "runner" in _cache:
        return _cache["runner"]
    import jax
    from jax.sharding import Mesh, NamedSharding, PartitionSpec
    from jax.experimental.shard_map import shard_map
    from concourse import mybir
    from concourse.bass2jax import (
        _bass_exec_p,
        install_neuronx_cc_hook,
        partition_id_tensor,
    )

    nc = _get_nc()
    install_neuronx_cc_hook()
    pname = nc.partition_id_tensor.name if nc.partition_id_tensor else None
    in_names, out_names, out_avals, zeros = [], [], [], []
    for alloc in nc.m.functions[0].allocations:
        if not isinstance(alloc, mybir.MemoryLocationSet):
            continue
        name = alloc.memorylocations[0].name
        if alloc.kind == "ExternalInput":
            if name != pname:
                in_names.append(name)
        elif alloc.kind == "ExternalOutput":
            shape = tuple(alloc.tensor_shape)
            dtype = mybir.dt.np(alloc.dtype)
            out_names.append(name)
            out_avals.append(jax.core.ShapedArray(shape, dtype))
            zeros.append(np.zeros(shape, dtype))
    names_all = in_names + out_names + ([pname] if pname else [])

    def _body(*args):
        operands = list(args)
        if pname is not None:
            operands.append(partition_id_tensor())
        return tuple(_bass_exec_p.bind(
            *operands,
            out_avals=tuple(out_avals),
            in_names=tuple(names_all),
            out_names=tuple(out_names),
            lowering_input_output_aliases=(),
            sim_require_finite=True,
            sim_require_nnan=True,
            nc=nc,
        ))

    devices = jax.devices()[:N_CORES]
    mesh = Mesh(np.asarray(devices), ("core",))
    nio = len(in_names) + len(out_names)
    sharded = jax.jit(
        shard_map(
            _body, mesh=mesh,
            in_specs=(PartitionSpec("core"),) * nio,
            out_specs=(PartitionSpec("core"),) * len(out_names),
            check_rep=False,
        ),
        keep_unused=True,
    )
    sh = NamedSharding(mesh, PartitionSpec("core"))
    zdev = [
        jax.device_put(np.zeros((N_CORES * z.shape[0], *z.shape[1:]), z.dtype), sh)
        for z in zeros
    ]
    _cache["runner"] = (sharded, in_names, out_names, out_avals, zdev, sh)
    return _cache["runner"]


def kernel(x, Wq, Wk, Wv, Wo, bo, trace=False):
    in_maps = make_in_maps(x, Wq, Wk, Wv, Wo, bo)
    try:
        import jax

        sharded, in_names, out_names, out_avals, zdev, sh = _get_runner()
        concat = [
            jax.device_put(
                np.concatenate([m[n] for m in in_maps], axis=0), sh
            )
            for n in in_names
        ]
        outs = sharded(*concat, *zdev)
        arr = np.asarray(outs[out_names.index("out")]).reshape(
            N_CORES, SEQ, EMBED
        )
        out = np.empty((BATCH, SEQ, EMBED), dtype=np.float32)
        for b in range(BATCH):
            out[b] = arr[2 * b] + arr[2 * b + 1]
        return out
    except Exception:
        from concourse.bass_utils import run_bass_kernel_spmd

        nc = _get_nc()
        res = run_bass_kernel_spmd(
            nc, in_maps, list(range(N_CORES)), trace=trace
        )
        _cache["last_result"] = res
        out = np.empty((BATCH, SEQ, EMBED), dtype=np.float32)
        for b in range(BATCH):
            out[b] = res.results[2 * b]["out"] + res.results[2 * b + 1]["out"]
        return out


# revision 7
# speedup vs baseline: 3.0187x; 1.0040x over previous
"""Multi-head attention (dense transformer block) on 8 TRN2 NeuronCores. v2

Sharding: 8 cores = 4 batches x 2 head-halves (as v1).
  core c: batch b = c // 2, head half H = c % 2 (heads H*8 .. H*8+8).
  Host sums core pairs; bias folded into the even core of each pair.

v2 datapath is bf16 end-to-end (x, Wq/Wk/Wv/Wo in bf16; psum f32):
  1. QK projections -> psum f32 -> DVE evac to QT/KT bf16 [d, s].
     V projection -> psum [s, d] -> vp bf16 [k, (si h d|1)] with a ones
     column per head (rowsum trick).
  2. scores: per (head, qj, ki): psum[k=128, 1024] = K^T x Q chunks,
     ACT exp (scale=1/8) -> et bf16 [128, 1024].
  3. attn@V in [q, d] orientation: stationary = et q-slice [k=128, q=128]
     (full PE utilization), moving = vp [k=128, 65]: out psum [q, 65]
     accumulated over ki; col 64 = softmax denominator (per-partition).
     Normalize = one DVE tensor_scalar divide; PE-transpose [q,64]->[64,q]
     via identity into outT rows (sub*64..) - no gpsimd broadcast, no
     cross-partition DMA staging.
  4. final: out[s,e] = sum_g outT[g]^T @ Wo[g]; bias added during the
     DVE psum evacuation (tensor_tensor add) instead of a K=1 matmul.

Engines consume their queues in order, so projection/final work is
emitted *woven between* attention ki-steps (pop_filler) - the exp chain
is ACT-bound and the PE would otherwise idle ~350ns per ki-step.
"""

from collections import deque

import numpy as np

EMBED = 1024
HEADS = 16
HEAD_DIM = 64
SEQ = 2048
BATCH = 4
N_CORES = 8

LOCAL_HEADS = 8
N_GROUPS = 4
WCOLS = LOCAL_HEADS * HEAD_DIM  # 512

P = 128
NS = SEQ // P    # 16
NE = EMBED // P  # 8
VB = HEAD_DIM + 1  # 65

_cache = {}


def _emit(nc, tc, tile, mybir, make_identity, d):
    f32 = mybir.dt.float32
    bf16 = mybir.dt.bfloat16
    EXP = mybir.ActivationFunctionType.Exp
    DIV = mybir.AluOpType.divide

    with (
        tc.tile_pool(name="const", bufs=1) as const_pool,
        tc.tile_pool(name="xt", bufs=1) as xt_pool,
        tc.tile_pool(name="v", bufs=2) as v_pool,
        tc.tile_pool(name="qk", bufs=2) as qk_pool,
        tc.tile_pool(name="wst", bufs=1) as wst_pool,
        tc.tile_pool(name="ps_s", bufs=2, space="PSUM") as ps_s,
        tc.tile_pool(name="ps_p", bufs=2, space="PSUM") as ps_p,
        tc.tile_pool(name="ps_o", bufs=1, space="PSUM") as ps_o,
    ):
        def load_wv(half):
            wvt = wst_pool.tile([P, NE * 256], bf16, tag="wv", bufs=2, name="wvt")
            wv_v = d["wv"][:].rearrange("(e p) c -> p e c", e=NE, p=P)
            nc.sync.dma_start(
                out=wvt[:].rearrange("p (e c) -> p e c", e=NE, c=256),
                in_=wv_v[:, :, half * 256:(half + 1) * 256],
            )
            return wvt

        def load_wqk(name, g):
            wt = wst_pool.tile([P, NE * P], bf16, tag="wqk", bufs=4, name="wqk")
            w_v = d[name][:].rearrange("(e p) c -> p e c", e=NE, p=P)
            nc.sync.dma_start(
                out=wt[:].rearrange("p (e c) -> p e c", e=NE, c=P),
                in_=w_v[:, :, g * P:(g + 1) * P],
            )
            return [wt[:, ei * P:(ei + 1) * P] for ei in range(NE)]

        # DMA queue order = need order: the g0 QK weights gate the first
        # scores, then the first xT s-slab, then the V weights. One DMA
        # per slab: each dma_start costs ~650ns of serialized DGE queue
        # time, so few big transfers beat many small ones.
        # xt layout is sj-major (sj, ei, 512) so each per-sj DMA writes one
        # FLAT 2-d span (a 3-d strided write region defeats subtile dep
        # tracking -> readers race the DMA). All reads stay within one sj.
        wqk_pre = {"wq": load_wqk("wq", 0), "wk": load_wqk("wk", 0)}
        xt_big = xt_pool.tile([P, NE * SEQ], bf16, tag="xt", name="xt_big")
        xt_in = d["xt"][:].rearrange("(e p) s -> p e s", e=NE, p=P)

        # sj0 is split into two flat half-slabs (ha, ei, 256) so the very
        # first projections gate on 0.5MB of DMA instead of 1MB.
        def load_xt_sj(sj):
            if sj == 0:
                for ha in range(2):
                    nc.sync.dma_start(
                        out=xt_big[:, ha * NE * 256:(ha + 1) * NE * 256],
                        in_=xt_in[:, :, ha * 256:(ha + 1) * 256],
                    )
                return
            nc.sync.dma_start(
                out=xt_big[:, sj * NE * 512:(sj + 1) * NE * 512],
                in_=xt_in[:, :, sj * 512:(sj + 1) * 512],
            )

        load_xt_sj(0)
        wvt_pre = load_wv(0)
        for sj in range(1, 4):
            load_xt_sj(sj)

        def xt_blk(ei, s0, slen):
            if s0 < 512:
                ha, off = divmod(s0, 256)
                assert off + slen <= 256, (s0, slen)
                base = (ha * NE + ei) * 256 + off
                return xt_big[:, base: base + slen]
            sj, off = s0 // 512, s0 % 512
            base = (sj * NE + ei) * 512 + off
            return xt_big[:, base: base + slen]

        ones128 = const_pool.tile([P, P], bf16, tag="ones", name="ones128")
        nc.gpsimd.memset(ones128[:], 1.0)
        ident = const_pool.tile([P, P], bf16, tag="ident", name="ident")
        make_identity(nc, ident[:])
        # warm the ACT exp table set during the DMA-bound startup
        warmf = const_pool.tile([1, 1], f32, tag="warmf", name="warmf")
        warm = const_pool.tile([1, 1], f32, tag="warm", name="warm")
        nc.vector.tensor_copy(warmf[:], ones128[0:1, 0:1])
        nc.scalar.activation(warm[:], warmf[:], EXP)
        # bias broadcast to all partitions (zeros on odd cores)
        bo_sb = const_pool.tile([1, EMBED], f32, tag="bo", name="bo_sb")
        nc.sync.dma_start(out=bo_sb[:], in_=d["bo"][:])
        bias_bc = const_pool.tile([P, EMBED], f32, tag="biasbc", name="bias_bc")
        nc.gpsimd.partition_broadcast(bias_bc[:], bo_sb[:])

        with (
            tc.tile_pool(name="et", bufs=6) as et_pool,
            tc.tile_pool(name="oq", bufs=3) as oq_pool,
            tc.tile_pool(name="outt", bufs=1) as outt_pool,
            tc.tile_pool(name="fin", bufs=4) as fin_pool,
        ):
            outt_tiles = [
                outt_pool.tile([P, SEQ], bf16, tag=f"outt{g}", name=f"outt{g}")
                for g in range(N_GROUPS)
            ]

            # vp: [128, NS*4*VB]; s-chunk si at si*4*VB, head (h%4) at h*VB;
            # col 64 of each head block is ones (rowsum trick).
            vp_tiles = []
            qkt_by_g = {}
            wo_tiles = []

            filler_q = deque()  # items: (tag, fn); tag marks the op's
            done_tags = set()   # completion point for ensure_tag()

            def pop_filler(n=1):
                for _ in range(n):
                    if filler_q:
                        tag, fn = filler_q.popleft()
                        fn()
                        if tag is not None:
                            done_tags.add(tag)

            def drain_fillers():
                pop_filler(len(filler_q))

            def ensure_tag(tag):
                while tag not in done_tags and filler_q:
                    pop_filler(1)

            def queue_ops(ops, tag=None):
                # tag attaches to the LAST op of the block
                for op in ops[:-1]:
                    filler_q.append((None, op))
                filler_q.append((tag, op if False else ops[-1]))

            def new_vp():
                vpt = v_pool.tile([P, NS * 4 * VB], bf16, tag="vp", name="vpt")
                vp_tiles.append(vpt)
                vp_v4 = vpt[:].rearrange("p (s h b) -> p s h b", s=NS, h=4, b=VB)
                nc.vector.tensor_copy(
                    vp_v4[:, :, :, HEAD_DIM:HEAD_DIM + 1],
                    ones128[:, 0:NS * 4].rearrange(
                        "p (a b c) -> p a b c", a=NS, b=4, c=1
                    ),
                )
                return vpt

            # Fillers are micro-ops (~one instruction each) so weaving them
            # into the ki-steps never delays the next scores matmul by more
            # than ~200ns (a chunky filler starves the ACT exp stream).
            def vblock_ops(wvt, vpt, si):
                vp_v = vpt[:].rearrange("p (s h b) -> p s h b", s=NS, h=4, b=VB)
                st = {}

                def mm(ei):
                    def go():
                        if ei == 0:
                            st["pt"] = ps_p.tile([P, 512], f32, tag="p", name="pt")
                        nc.tensor.matmul(
                            st["pt"][:, 0:256],
                            xt_blk(ei, si * P, P),
                            wvt[:, ei * 256:(ei + 1) * 256],
                            start=(ei == 0),
                            stop=(ei == NE - 1),
                        )
                    return go

                def evac():
                    nc.vector.tensor_copy(
                        vp_v[:, si, :, 0:HEAD_DIM],
                        st["pt"][:, 0:256].rearrange(
                            "p (h b) -> p h b", h=4, b=HEAD_DIM
                        ),
                    )
                return [mm(ei) for ei in range(NE)] + [evac]

            def qkchunk_ops(wtiles, dst, sj):
                # sj0 reads the split xt half-slabs: two 256-wide moving
                # passes per ei (start=True once arms the bank; the other
                # regions zero-fill via pending-zero on first write).
                st = {}
                pieces = [(0, 256), (256, 256)] if sj == 0 else [(0, 512)]

                def mm(ei, off, w):
                    def go():
                        if ei == 0 and off == 0:
                            st["pt"] = ps_p.tile([P, 512], f32, tag="p", name="pt")
                        nc.tensor.matmul(
                            st["pt"][:, off:off + w],
                            wtiles[ei],
                            xt_blk(ei, sj * 512 + off, w),
                            start=(ei == 0 and off == 0),
                            stop=(ei == NE - 1 and off + w == 512),
                            skip_group_check=True,
                        )
                    return go

                def evac():
                    nc.vector.tensor_copy(
                        dst[:, sj * 512:(sj + 1) * 512], st["pt"][:, 0:512]
                    )
                ops = []
                for off, w in pieces:
                    ops.extend(mm(ei, off, w) for ei in range(NE))
                return ops + [evac]

            def final_ops(si, tail=False, force_s=False):
                st = {}
                ops = []

                def mm(ej, c):
                    def go():
                        if c == 0:
                            if ej == 0:
                                st["ot"] = fin_pool.tile(
                                    [P, 1024], f32, tag="ot", name="ot"
                                )
                            if force_s or (tail and (si + ej) % 2 == 1):
                                st[ej] = ps_s.tile([P, 1024], f32, tag="s", name="pt")
                            else:
                                st[ej] = ps_p.tile([P, 512], f32, tag="p", name="pt")
                        nc.tensor.matmul(
                            st[ej][:, 0:512],
                            outt_tiles[c][:, si * P:(si + 1) * P],
                            wo_tiles[c // 2][:, (c % 2) * 1024 + ej * 512:
                                             (c % 2) * 1024 + (ej + 1) * 512],
                            start=(c == 0),
                            stop=(c == 3),
                            skip_group_check=True,
                        )
                    return go

                def evac(ej):
                    def go():
                        nc.vector.tensor_tensor(
                            out=st["ot"][:, ej * 512:(ej + 1) * 512],
                            in0=st[ej][:, 0:512],
                            in1=bias_bc[:, ej * 512:(ej + 1) * 512],
                            op=mybir.AluOpType.add,
                        )
                    return go

                def dma():
                    nc.sync.dma_start(
                        out=d["out"][si * P:(si + 1) * P, :], in_=st["ot"][:]
                    )
                for ej in range(2):
                    ops.extend([mm(ej, c) for c in range(4)])
                    ops.append(evac(ej))
                ops.append(dma)
                return ops

            def run_all(ops):
                for op in ops:
                    op()

            def load_wo():
                wo_a = wst_pool.tile([P, SEQ], bf16, tag="wv", bufs=2, name="wo_a")
                wo_b = qk_pool.tile([P, SEQ], bf16, tag="wqt", name="wo_b")
                wo_tiles.extend([wo_a, wo_b])
                for j in range(2):
                    for jj in range(2):
                        c = 2 * j + jj
                        nc.sync.dma_start(
                            out=wo_tiles[j][:, jj * 1024:(jj + 1) * 1024],
                            in_=d["wo"][c * P:(c + 1) * P, :],
                        )

            def vp_blk(si, h):
                vpt = vp_tiles[h // 4]
                base = (si * 4 + (h % 4)) * VB
                return vpt[:, base: base + VB]

            # ---- prelude: g0 QT/KT direct (gates the exp stream), then
            # the first V' blocks; the rest of V' weaves in as fillers.
            vpt0 = new_vp()
            qkt0 = {}
            for name in ("wq", "wk"):
                dst = qk_pool.tile([P, SEQ], bf16, tag=f"{name}t", name=f"{name}t")
                qkt0[name] = dst
            qkt_by_g[0] = qkt0
            # All g0 projection work is queued (not run) in rough need
            # order; unit 0's ki-loop pulls it just-in-time via ensure_tag
            # so the exp stream starts ~20us earlier than a serial prelude.
            TN = {"wq": "qt", "wk": "kt"}

            def q_qk0(name, sj):
                queue_ops(
                    qkchunk_ops(wqk_pre[name], qkt0[name], sj),
                    tag=(TN[name], 0, sj),
                )

            def q_vp0(si):
                queue_ops(vblock_ops(wvt_pre, vpt0, si), tag=("vp", 0, si))

            q_qk0("wk", 0)
            q_qk0("wq", 0)
            q_qk0("wq", 1)
            for si in range(4):
                q_vp0(si)
            q_qk0("wk", 1)
            for si in range(4, 7):
                q_vp0(si)
            q_qk0("wk", 2)
            for si in range(7, 10):
                q_vp0(si)
            q_qk0("wk", 3)
            for si in range(10, NS):
                q_vp0(si)
            q_qk0("wq", 2)
            q_qk0("wq", 3)

            # ---- groups ------------------------------------------------
            # Unit epilogue is split: the 8 DVE divides are emitted at unit
            # end (freeing the po banks before the next unit's attnV), while
            # the PE transposes + outT copies defer to ki==2 of the next
            # unit so the PE never parks on the divide chain's latency.
            pending_tr = [None]

            def emit_divides(po_q):
                oqs = []
                for qi in range(8):
                    rec = oq_pool.tile([P, 1], f32, tag="rec", bufs=9, name="rec")
                    nc.vector.reciprocal(rec[:], po_q(qi)[:, HEAD_DIM:VB])
                    oq = oq_pool.tile(
                        [P, HEAD_DIM], bf16, tag="oq", bufs=9, name="oq"
                    )
                    oqs.append(oq)
                    nc.vector.tensor_scalar(
                        oq[:],
                        po_q(qi)[:, 0:HEAD_DIM],
                        rec[:],
                        None,
                        op0=mybir.AluOpType.mult,
                    )
                return oqs

            def mk_trs(g, sub, q0, oqs):
                rows = slice(sub * HEAD_DIM, (sub + 1) * HEAD_DIM)

                def go():
                    for half4 in range(2):
                        tr = ps_p.tile([P, 512], bf16, tag="p", name="tr")
                        for j in range(4):
                            nc.tensor.transpose(
                                tr[rows, j * P:(j + 1) * P],
                                oqs[half4 * 4 + j][:],
                                ident[:],
                            )
                        nc.vector.tensor_copy(
                            outt_tiles[g][rows,
                                          q0 + half4 * 512: q0 + (half4 + 1) * 512],
                            tr[rows, :],
                        )
                return go

            vpt1_holder = {}
            deferred = {g: [] for g in range(N_GROUPS)}
            for g in range(N_GROUPS):
                # Stage the NEXT group's startup-critical chunks (first kt,
                # the qj=0 qt chunks, first V' blocks) into this group's
                # filler stream; the rest of its projections defer into its
                # OWN phase and are pulled just-in-time via ensure_tag -
                # this balances filler inventory against each phase's
                # ACT-bound PE holes (g2/g3 otherwise starve while g0/g1
                # burst-drain).
                if g + 1 < N_GROUPS:
                    gn = g + 1
                    if gn % 2 == 0:
                        wvt_n = load_wv(1)
                        vpt1 = new_vp()
                        vpt1_holder["wvt"] = wvt_n
                        vpt1_holder["vpt"] = vpt1
                        for si in range(NS):
                            queue_ops(vblock_ops(wvt_n, vpt1, si), tag=("vp", 1, si))
                    qkt_n = {}
                    for name in ("wq", "wk"):
                        dst = qk_pool.tile(
                            [P, SEQ], bf16, tag=f"{name}t", name=f"{name}t"
                        )
                        qkt_n[name] = dst
                        wtiles = load_wqk(name, gn)
                        for sj in range(4):
                            queue_ops(
                                qkchunk_ops(wtiles, dst, sj),
                                tag=({"wq": "qt", "wk": "kt"}[name], gn, sj),
                            )
                    qkt_by_g[gn] = qkt_n
                else:
                    load_wo()

                qt = qkt_by_g[g]["wq"]
                kt = qkt_by_g[g]["wk"]

                # Group 3 runs q-half-major; final si 0..7 only become
                # emittable once both qj=0 norms are in the stream (unit
                # index 2), so their fillers are queued there.
                if g < 3:
                    hq = [(s, qj) for s in range(2) for qj in range(2)]
                else:
                    hq = [(s, qj) for qj in range(2) for s in range(2)]
                for u, (sub, qj) in enumerate(hq):
                    h = 2 * g + sub
                    dr = slice(sub * HEAD_DIM, (sub + 1) * HEAD_DIM)
                    q0 = qj * 1024
                    po_a = ps_o.tile([P, 4 * VB], f32, tag="o0", name="po_a")
                    po_b = ps_o.tile([P, 4 * VB], f32, tag="o1", name="po_b")

                    def po_q(qi, _a=po_a, _b=po_b):
                        t = _a if qi < 4 else _b
                        return t[:, (qi % 4) * VB:(qi % 4) * VB + VB]

                    ensure_tag(("qt", g, 2 * qj))
                    ensure_tag(("qt", g, 2 * qj + 1))
                    et_tiles = {}
                    for ki in range(NS):
                        ensure_tag(("kt", g, ki // 4))
                        sps = ps_s.tile([P, 1024], f32, tag="s", name="sps")
                        for hf in range(2):
                            nc.tensor.matmul(
                                sps[:, hf * 512:(hf + 1) * 512],
                                kt[dr, ki * P:(ki + 1) * P],
                                qt[dr, q0 + hf * 512: q0 + (hf + 1) * 512],
                                start=True,
                                stop=True,
                            )
                        et = et_pool.tile([P, 1024], bf16, tag="et", name="et")
                        et_tiles[ki] = et
                        nc.scalar.activation(et[:], sps[:], EXP, scale=1.0 / 8.0)
                        # attnV skewed three ki behind the exp stream so the
                        # in-order PE queue never parks on a pending exp
                        if ki >= 3:
                            ensure_tag(("vp", h // 4, ki - 3))
                            for qi in range(8):
                                # start=True arms zero-on-write for the WHOLE
                                # 2KB psum bank, so only the first region per
                                # bank may set it; the others zero-fill via
                                # the armed pending-zero on their first write.
                                nc.tensor.matmul(
                                    po_q(qi),
                                    et_tiles[ki - 3][:, qi * P:(qi + 1) * P],
                                    vp_blk(ki - 3, h),
                                    start=(ki == 3 and qi % 4 == 0),
                                    stop=False,
                                    skip_group_check=True,
                                )
                        if ki == 2:
                            if pending_tr[0] is not None:
                                pending_tr[0]()
                                pending_tr[0] = None
                            # final si 0..7 become emittable once both qj=0
                            # outT halves are in the stream (g3 unit 2)
                            if g == 3 and u == 2:
                                for si in range(8):
                                    queue_ops(final_ops(si))
                        pop_filler(4 if g == 1 else 3)
                    ensure_tag(("vp", h // 4, NS - 1))
                    for kl in (NS - 3, NS - 2, NS - 1):
                        for qi in range(8):
                            nc.tensor.matmul(
                                po_q(qi),
                                et_tiles[kl][:, qi * P:(qi + 1) * P],
                                vp_blk(kl, h),
                                start=False,
                                stop=(kl == NS - 1),
                                skip_group_check=True,
                            )
                    oqs = emit_divides(po_q)
                    pending_tr[0] = mk_trs(g, sub, q0, oqs)

                if g < 3:
                    drain_fillers()

            # The first two tail finals' c0..c2 accumulations depend only
            # on outT[0..2]; emitting them before the last unit's transposes
            # keeps the PE fed while the divide chain drains on DVE.
            # ops layout per si: [mm(ej0,c0..c3), evac0, mm(ej1,c0..c3),
            # evac1, dma]; only the c3 matmuls read outT[3], so c0..c2 of
            # both ej may run before the last transposes land.
            head_a = final_ops(8, tail=True, force_s=True)
            for op in head_a[0:3] + head_a[5:8]:
                op()
            pending_tr[0]()
            drain_fillers()
            for op in head_a[3:5] + head_a[8:11]:
                op()
            for si in range(9, NS):
                run_all(final_ops(si, tail=True))


def _build_nc():
    import concourse.mybir as mybir
    import concourse.tile as tile
    from concourse import bacc
    from concourse.masks import make_identity

    f32 = mybir.dt.float32
    bf16 = mybir.dt.bfloat16
    nc = bacc.Bacc(
        "TRN2", target_bir_lowering=False, debug=False, num_devices=N_CORES
    )
    d = {
        "xt": nc.dram_tensor("xt", [EMBED, SEQ], bf16, kind="ExternalInput"),
        "wq": nc.dram_tensor("wq", [EMBED, WCOLS], bf16, kind="ExternalInput"),
        "wk": nc.dram_tensor("wk", [EMBED, WCOLS], bf16, kind="ExternalInput"),
        "wv": nc.dram_tensor("wv", [EMBED, WCOLS], bf16, kind="ExternalInput"),
        "wo": nc.dram_tensor("wo", [WCOLS, EMBED], bf16, kind="ExternalInput"),
        "bo": nc.dram_tensor("bo", [1, EMBED], f32, kind="ExternalInput"),
        "out": nc.dram_tensor("out", [SEQ, EMBED], f32, kind="ExternalOutput"),
    }
    with tile.TileContext(nc) as tc:
        _emit(nc, tc, tile, mybir, make_identity, d)
    nc.compile()
    return nc


def _get_nc():
    if "nc" not in _cache:
        _cache["nc"] = _build_nc()
    return _cache["nc"]


def make_in_maps(x, Wq, Wk, Wv, Wo, bo):
    import ml_dtypes

    bfarr = lambda a: np.ascontiguousarray(
        np.asarray(a, np.float32).astype(ml_dtypes.bfloat16)
    )
    x = np.asarray(x, dtype=np.float32)
    bo = np.asarray(bo, dtype=np.float32)
    xts = [bfarr(x[b].T) for b in range(BATCH)]
    Wq = np.asarray(Wq, np.float32)
    Wk = np.asarray(Wk, np.float32)
    Wv = np.asarray(Wv, np.float32)
    Wo = np.asarray(Wo, np.float32)
    in_maps = []
    for c in range(N_CORES):
        b, H = c // 2, c % 2
        cs = slice(H * WCOLS, (H + 1) * WCOLS)
        bo_eff = bo if H == 0 else np.zeros_like(bo)
        in_maps.append({
            "xt": xts[b],
            "wq": bfarr(Wq[:, cs]),
            "wk": bfarr(Wk[:, cs]),
            "wv": bfarr(Wv[:, cs]),
            "wo": bfarr(Wo[cs, :]),
            "bo": np.ascontiguousarray(bo_eff.reshape(1, EMBED)),
        })
    return in_maps


def _get_runner():
    """Cached jitted SPMD callable (avoids per-call retrace)."""
    if "runner" in _cache:
        return _cache["runner"]
    import jax
    from jax.sharding import Mesh, NamedSharding, PartitionSpec
    from jax.experimental.shard_map import shard_map
    from concourse import mybir
    from concourse.bass2jax import (
        _bass_exec_p,
        install_neuronx_cc_hook,
        partition_id_tensor,
    )

    nc = _get_nc()
    install_neuronx_cc_hook()
    pname = nc.partition_id_tensor.name if nc.partition_id_tensor else None
    in_names, out_names, out_avals, zeros = [], [], [], []
    for alloc in nc.m.functions[0].allocations:
        if not isinstance(alloc, mybir.MemoryLocationSet):
            continue
        name = alloc.memorylocations[0].name
        if alloc.kind == "ExternalInput":
            if name != pname:
                in_names.append(name)
        elif alloc.kind == "ExternalOutput":
            shape = tuple(alloc.tensor_shape)
            dtype = mybir.dt.np(alloc.dtype)
            out_names.append(name)
            out_avals.append(jax.core.ShapedArray(shape, dtype))
            zeros.append(np.zeros(shape, dtype))
    names_all = in_names + out_names + ([pname] if pname else [])

    def _body(*args):
        operands = list(args)
        if pname is not None:
            operands.append(partition_id_tensor())
        return tuple(_bass_exec_p.bind(
            *operands,
            out_avals=tuple(out_avals),
            in_names=tuple(names_all),
            out_names=tuple(out_names),
            lowering_input_output_aliases=(),
            sim_require_finite=True,
            sim_require_nnan=True,
            nc=nc,
        ))

    devices = jax.devices()[:N_CORES]
    mesh = Mesh(np.asarray(devices), ("core",))
    nio = len(in_names) + len(out_names)
    sharded = jax.jit(
        shard_map(
            _body, mesh=mesh,
            in_specs=(PartitionSpec("core"),) * nio,
            out_specs=(PartitionSpec("core"),) * len(out_names),
            check_rep=False,
        ),
        keep_unused=True,
    )
    sh = NamedSharding(mesh, PartitionSpec("core"))
    zdev = [
        jax.device_put(np.zeros((N_CORES * z.shape[0], *z.shape[1:]), z.dtype), sh)
        for z in zeros
    ]
    _cache["runner"] = (sharded, in_names, out_names, out_avals, zdev, sh)
    return _cache["runner"]


def kernel(x, Wq, Wk, Wv, Wo, bo, trace=False):
    in_maps = make_in_maps(x, Wq, Wk, Wv, Wo, bo)
    try:
        import jax

        sharded, in_names, out_names, out_avals, zdev, sh = _get_runner()
        concat = [
            jax.device_put(
                np.concatenate([m[n] for m in in_maps], axis=0), sh
            )
            for n in in_names
        ]
        outs = sharded(*concat, *zdev)
        arr = np.asarray(outs[out_names.index("out")]).reshape(
            N_CORES, SEQ, EMBED
        )
        out = np.empty((BATCH, SEQ, EMBED), dtype=np.float32)
        for b in range(BATCH):
            out[b] = arr[2 * b] + arr[2 * b + 1]
        return out
    except Exception:
        from concourse.bass_utils import run_bass_kernel_spmd

        nc = _get_nc()
        res = run_bass_kernel_spmd(
            nc, in_maps, list(range(N_CORES)), trace=trace
        )
        _cache["last_result"] = res
        out = np.empty((BATCH, SEQ, EMBED), dtype=np.float32)
        for b in range(BATCH):
            out[b] = res.results[2 * b]["out"] + res.results[2 * b + 1]["out"]
        return out
